# revision 9
# baseline (speedup 1.0000x reference)
"""Trainium2 Bass kernel for nn_BandwidthConstrainedComm.

GNN message passing: per batch element, N=256 agents each generate a
message (MLP -> compress -> decompress), compute pairwise bilinear
relevance scores, pick top-K=8 senders (softmax gated), aggregate their
messages, and run a receiver MLP over [obs, agg].

Sharding: pure data parallel over batch B=128 -> 16 per core x 8 cores.

Design notes (v2 - 5-stage software pipeline):
  - all inputs pre-cast to bf16 on the host; obs staged as [D, bpc, N]
    so every DMA line is a dense 1KB segment.
  - W2@Wc@Wd fused into one [H1, MSG] matrix on the host.
  - message bias bf = b2@Wc@Wd + bc@Wd + bd folded through the
    aggregation (gates sum to 1) into the receiver matmul via a
    ones-row in aggT and a host-precomputed (bf@Wr1c + br1) row
    appended to Wr1c -- br1 rides along for free, so the receiver
    relu needs no bias and runs as ONE [128,1024] activation.
  - br2 added on the host after gathering.
  - top-8 via DVE Max8 over exp'd scores; U = (E >= t8)*E via
    scalar_tensor_tensor on GPSIMD (idle engine), den = sum(top8) via
    gpsimd tensor_reduce, diag(1/den) built on gpsimd from a persistent
    identity. Only Max8 + one tiny reciprocal stay on DVE.
  - gate transpose+normalize in one regular matmul per 128-chunk:
    Gt = U.T @ diag(1/den).
  - per-pair work is split into FIVE pipeline stages emitted across
    iterations (pre(k) | gate(k) | Gt+aggT(k-2) | l1(k-3) | l2+out(k-4))
    so every cross-engine dependency has >= half an iteration of slack
    and the in-order tensor queue never head-of-line blocks.
  - engine load balance per pair (~5us each): PE all matmuls; scalar
    relu_h, out-cast, exp x2, relu_r, aggT-copy; vector tmpT-copy,
    msgs-copy, Gt-casts, Max8 x4, recip; gpsimd STT/den/diag x4.
  - PSUM: 4 tags / 14KB of 16KB: g[2KB x1]=hT->msn, a[4KB x1]=tmp->r,
    b[4KB x1]=o->s, c[2KB x2]=Gt0,Gt1,aggT (+warmup).
  - output written as bf16 [D, bpc, N] (dense lines), un-transposed and
    f32-cast on the host.
"""

import sys

sys.path.insert(0, "/opt/trn_rl_repo")

import numpy as np

# problem dims (hardcoded per contract)
B, N, D = 128, 256, 256
MSG, CD, K = 64, 32, 8
H1, H2 = 128, 256
NCORES = 8
BPC = B // NCORES  # batches per core

_CACHE = {}


def build_program(bpc=BPC, passes=1):
    import concourse.bacc as bacc
    import concourse.mybir as mybir
    import concourse.tile as tile
    from concourse.masks import make_identity
    from contextlib import ExitStack

    dt = mybir.dt
    f32, bf16 = dt.float32, dt.bfloat16
    AF = mybir.ActivationFunctionType
    OP = mybir.AluOpType

    assert bpc % 2 == 0
    npairs = bpc // 2

    nc = bacc.Bacc("TRN2", target_bir_lowering=False, debug=False,
                   num_devices=NCORES)

    obsT_d = nc.dram_tensor("obsT", [D, bpc, N], bf16, kind="ExternalInput")
    W1_d = nc.dram_tensor("W1", [D, H1], bf16, kind="ExternalInput")
    Wf_d = nc.dram_tensor("Wf", [H1, MSG], bf16, kind="ExternalInput")
    Wbil_d = nc.dram_tensor("Wbil", [D, D], bf16, kind="ExternalInput")
    Wr1a_d = nc.dram_tensor("Wr1a", [D, H2], bf16, kind="ExternalInput")
    Wr1c_d = nc.dram_tensor("Wr1c", [MSG + 1, H2], bf16,
                            kind="ExternalInput")
    Wr2_d = nc.dram_tensor("Wr2", [H2, D], bf16, kind="ExternalInput")
    b1_d = nc.dram_tensor("b1", [H1], f32, kind="ExternalInput")
    out_d = nc.dram_tensor("out", [D, bpc, N], bf16, kind="ExternalOutput")

    with tile.TileContext(nc) as tc, ExitStack() as ctx:
        wp = ctx.enter_context(tc.tile_pool(name="wp", bufs=1))
        dp = ctx.enter_context(tc.tile_pool(name="dp", bufs=5))
        sp = ctx.enter_context(tc.tile_pool(name="sp", bufs=3))
        pp = ctx.enter_context(tc.tile_pool(name="pp", bufs=1, space="PSUM"))

        # ---------------- one-time setup ----------------
        # warmup burst: dense PE work on a junk tile during the initial
        # DMA latency so the HAM clock-gate is at 8/8 when the first
        # real matmuls land.
        junk = wp.tile([128, 128], bf16, name="junk")
        nc.vector.memset(junk[:], 0.25)
        warm_ps = pp.tile([128, 128], f32, tag="c", bufs=2)
        for _ in range(16):
            nc.tensor.matmul(warm_ps[:], junk[:], junk[:],
                             start=True, stop=True)
        warm_sink = wp.tile([1, 8], f32, name="warm_sink")
        nc.vector.tensor_copy(warm_sink[:], warm_ps[0:1, 0:8])

        ident = wp.tile([128, 128], f32)
        make_identity(nc, ident[:])
        ident_b = wp.tile([128, 128], bf16)
        nc.vector.tensor_copy(ident_b[:], ident[:])

        def loadw(dram_ap, shape, name, eng=nc.scalar):
            t = wp.tile(shape, bf16, name=name)
            eng.dma_start(t[:], dram_ap)
            return t

        W1_r0 = loadw(W1_d[0:128, :], [128, H1], "W1a")
        W1_r1 = loadw(W1_d[128:256, :], [128, H1], "W1b", nc.gpsimd)
        Wf_b = loadw(Wf_d[:], [H1, MSG], "Wf")
        Wb_r0 = loadw(Wbil_d[0:128, :], [128, D], "Wba", nc.gpsimd)
        Wb_r1 = loadw(Wbil_d[128:256, :], [128, D], "Wbb")
        Wr1_r0 = loadw(Wr1a_d[0:128, :], [128, H2], "Wr1a", nc.gpsimd)
        Wr1_r1 = loadw(Wr1a_d[128:256, :], [128, H2], "Wr1b")
        Wr1c_b = loadw(Wr1c_d[:], [MSG + 1, H2], "Wr1c", nc.gpsimd)
        Wr2_r0 = loadw(Wr2_d[0:128, :], [128, D], "Wr2a")
        Wr2_r1 = loadw(Wr2_d[128:256, :], [128, D], "Wr2b", nc.gpsimd)

        b1_sb = wp.tile([H1, 1], f32, name="b1s")
        nc.scalar.dma_start(
            b1_sb[:], b1_d[:].rearrange("(p o) -> p o", o=1))

        # persistent aggT tiles with a constant ones-row (row MSG) for
        # the folded message bias (+ br1)
        aggT_tiles = []
        for i in range(2):
            t = wp.tile([MSG + 1, 2, N], bf16, name=f"aggTp{i}")
            nc.vector.memset(t[MSG:MSG + 1, :, :], 1.0)
            aggT_tiles.append(t)

        # ---------------- pipeline stages ----------------
        state = {}

        def emit_od(p):
            od_b = []
            for dc in range(2):
                ob = dp.tile([128, 2, N], bf16, name=f"od{dc}",
                             tag=f"od{dc}", bufs=5)
                nc.sync.dma_start(
                    ob[:], obsT_d[128 * dc:128 * (dc + 1),
                                  2 * p:2 * p + 2, :])
                od_b.append(ob[:].rearrange("d b n -> d (b n)"))
            state[("od", p)] = od_b

        def emit_ht(p):
            od_b = state[("od", p)]
            hT_ps = pp.tile([H1, 2 * N], f32, tag="g", bufs=1)
            nc.tensor.matmul(hT_ps[:], W1_r0[:], od_b[0],
                             start=True, stop=False)
            nc.tensor.matmul(hT_ps[:], W1_r1[:], od_b[1],
                             start=False, stop=True)
            hT_b = sp.tile([H1, 2 * N], bf16, name="hT_b", tag="hT",
                           bufs=2)
            nc.scalar.activation(hT_b[:], hT_ps[:], AF.Relu, bias=b1_sb[:])
            state[("hT", p)] = hT_b

        def emit_tmp(p):
            od_b = state[("od", p)]
            tmp_ps = pp.tile([128, 2, 2 * N], f32, tag="a", bufs=1)
            for ec in range(2):
                nc.tensor.matmul(tmp_ps[:, ec, :],
                                 Wb_r0[:, 128 * ec:128 * (ec + 1)],
                                 od_b[0], start=True, stop=False)
                nc.tensor.matmul(tmp_ps[:, ec, :],
                                 Wb_r1[:, 128 * ec:128 * (ec + 1)],
                                 od_b[1], start=False, stop=True)
            tmpT_r = sp.tile([128, 2, 2 * N], bf16, name="tmpT_r",
                             tag="tmpT", bufs=2)
            nc.vector.tensor_copy(
                tmpT_r[:].rearrange("e c f -> e (c f)"),
                tmp_ps[:].rearrange("e c f -> e (c f)"))
            state[("tmpT", p)] = tmpT_r

        def emit_msn(p):
            hT_b = state.pop(("hT", p))
            msn_ps = pp.tile([128, 4, MSG], f32, tag="g", bufs=1,
                             name="msn_ps")
            for q in range(4):
                nc.tensor.matmul(msn_ps[:, q, :],
                                 hT_b[:, 128 * q:128 * (q + 1)],
                                 Wf_b[:], start=True, stop=True)
            msgs_b = sp.tile([128, 4, MSG], bf16, name="msgs_b",
                             tag="msgs", bufs=3)
            nc.vector.tensor_copy(msgs_b[:], msn_ps[:])
            state[("msgs", p)] = msgs_b

        def emit_scores(p):
            od_b = state[("od", p)]
            tmpT_r = state.pop(("tmpT", p))
            s_ps = pp.tile([128, 2, 2, N], f32, tag="b", bufs=1,
                           name="s_ps")
            for bi in range(2):
                boff = bi * N
                for ic in range(2):
                    ioff = boff + 128 * ic
                    nc.tensor.matmul(s_ps[:, bi, ic, :],
                                     tmpT_r[:, 0, ioff:ioff + 128],
                                     od_b[0][:, boff:boff + N],
                                     start=True, stop=False)
                    nc.tensor.matmul(s_ps[:, bi, ic, :],
                                     tmpT_r[:, 1, ioff:ioff + 128],
                                     od_b[1][:, boff:boff + N],
                                     start=False, stop=True)
            Es = []
            for bi in range(2):
                E = sp.tile([128, 2, N], bf16, name="E", tag="E", bufs=4)
                nc.scalar.activation(
                    E[:].rearrange("p c f -> p (c f)"),
                    s_ps[:, bi].rearrange("p c f -> p (c f)"), AF.Exp)
                Es.append(E)
            state[("E", p)] = Es

        def emit_gate(p):
            # top-8 + den on DVE, then normalized gates on GPSIMD:
            #   msk = (E >= t8) * rden   (two per-partition scalars)
            #   U   = msk * E            (already softmax-normalized)
            Es = state.pop(("E", p))
            den = sp.tile([128, 4], f32, name="den", tag="den", bufs=3)
            rden = sp.tile([128, 4], f32, name="rden", tag="rden", bufs=3)
            top8s = []
            for bi in range(2):
                for ic in range(2):
                    c = 2 * bi + ic
                    top8 = sp.tile([128, 8], f32, name="top8",
                                   tag="top8", bufs=8)
                    nc.vector.max(out=top8[:], in_=Es[bi][:, ic, :])
                    nc.vector.tensor_reduce(
                        out=den[:, c:c + 1], in_=top8[:],
                        axis=mybir.AxisListType.X, op=OP.add)
                    top8s.append(top8)
            nc.vector.reciprocal(rden[:], den[:])
            Us = []
            for bi in range(2):
                U = sp.tile([128, 2, N], bf16, name="U", tag=f"U{bi}",
                            bufs=3)
                for ic in range(2):
                    c = 2 * bi + ic
                    msk = sp.tile([128, N], bf16, name="msk", tag="msk",
                                  bufs=4)
                    nc.gpsimd.tensor_scalar(
                        out=msk[:], in0=Es[bi][:, ic, :],
                        scalar1=top8s[c][:, 7:8],
                        scalar2=rden[:, c:c + 1],
                        op0=OP.is_ge, op1=OP.mult)
                    nc.gpsimd.tensor_tensor(
                        out=U[:, ic, :], in0=msk[:],
                        in1=Es[bi][:, ic, :], op=OP.mult)
                Us.append(U)
            state[("gate", p)] = Us

        def emit_gt(p):
            Us = state.pop(("gate", p))
            Gt_bs = []
            for bi in range(2):
                U = Us[bi]
                Gt_ps = pp.tile([128, 2, N], f32, tag="c", bufs=2,
                                name="Gt_ps")
                for ic in range(2):
                    for jc in range(2):
                        nc.tensor.matmul(
                            Gt_ps[:, jc, 128 * ic:128 * (ic + 1)],
                            U[:, ic, 128 * jc:128 * (jc + 1)],
                            ident_b[:], start=True, stop=True)
                Gt_b = sp.tile([128, 2, N], bf16, name="Gt_b", tag="Gt",
                               bufs=4)
                nc.vector.tensor_copy(
                    Gt_b[:].rearrange("p c f -> p (c f)"),
                    Gt_ps[:].rearrange("p c f -> p (c f)"))
                Gt_bs.append(Gt_b)
            state[("Gt", p)] = Gt_bs

        def emit_agg(p):
            Gt_bs = state.pop(("Gt", p))
            msgs_b = state.pop(("msgs", p))
            aggT_ps = pp.tile([MSG, 2, N], f32, tag="c", bufs=2,
                              name="aggT_ps")
            for bi in range(2):
                nc.tensor.matmul(aggT_ps[:, bi, :],
                                 msgs_b[:, 2 * bi, :], Gt_bs[bi][:, 0, :],
                                 start=True, stop=False)
                nc.tensor.matmul(aggT_ps[:, bi, :],
                                 msgs_b[:, 2 * bi + 1, :],
                                 Gt_bs[bi][:, 1, :],
                                 start=False, stop=True)
            aggT_r = aggT_tiles[p % 2]
            nc.scalar.activation(
                aggT_r[0:MSG, :, :].rearrange("m b n -> m (b n)"),
                aggT_ps[:].rearrange("m b n -> m (b n)"), AF.Copy)

        def emit_l1(p):
            od_b = state[("od", p)]
            aggT_r = aggT_tiles[p % 2]
            r_ps = pp.tile([128, 2, 2 * N], f32, tag="a", bufs=1,
                           name="r_ps")
            aggT_ap = aggT_r[:].rearrange("m b n -> m (b n)")
            for mi in range(2):
                ms = 128 * mi
                nc.tensor.matmul(r_ps[:, mi, :], Wr1_r0[:, ms:ms + 128],
                                 od_b[0], start=True, stop=False)
                nc.tensor.matmul(r_ps[:, mi, :], Wr1_r1[:, ms:ms + 128],
                                 od_b[1], start=False, stop=False)
                nc.tensor.matmul(r_ps[:, mi, :], Wr1c_b[:, ms:ms + 128],
                                 aggT_ap, start=False, stop=True)
            rT = sp.tile([128, 2, 2 * N], bf16, name="rT", tag="rT",
                         bufs=3)
            nc.scalar.activation(
                rT[:].rearrange("h c f -> h (c f)"),
                r_ps[:].rearrange("h c f -> h (c f)"), AF.Relu)
            state[("rT", p)] = rT

        def emit_l2(p):
            rT = state.pop(("rT", p))
            state.pop(("od", p))
            o_ps = pp.tile([128, 2, 2 * N], f32, tag="b", bufs=1,
                           name="o_ps")
            for dc in range(2):
                ds = 128 * dc
                nc.tensor.matmul(o_ps[:, dc, :], Wr2_r0[:, ds:ds + 128],
                                 rT[:, 0, :], start=True, stop=False)
                nc.tensor.matmul(o_ps[:, dc, :], Wr2_r1[:, ds:ds + 128],
                                 rT[:, 1, :], start=False, stop=True)
            o_sb = sp.tile([128, 2, 2, N], bf16, name="o_sb", tag="o_sb",
                           bufs=2)
            nc.scalar.activation(
                o_sb[:].rearrange("d c b n -> d (c b n)"),
                o_ps[:].rearrange("d c f -> d (c f)"), AF.Copy)
            b0 = 2 * p
            for dc in range(2):
                nc.sync.dma_start(
                    out_d[128 * dc:128 * (dc + 1), b0:b0 + 2, :],
                    o_sb[:, dc])

        # ---------------- main pipeline loop ----------------
        # Per-iteration emission order fixes each engine's queue order:
        #   PE:     hT(v) tmp(v) l2(v-4) msn(v) Gt(v-2) s(v) l1(v-3)
        #           aggT(v-2)
        #   scalar: relu_h(v) out(v-4) exp(v) relu_r(v-3) aggT-cp(v-2)
        #   vector: tmpT(v) msgs(v) Gt-cast(v-2) Max8/recip(v)
        #   gpsimd: STT/den(v) diag(v)
        for _ in range(passes):
            emit_od(0)
            for v in range(npairs + 4):
                if v + 1 < npairs:
                    emit_od(v + 1)
                if v < npairs:
                    emit_ht(v)
                    emit_tmp(v)
                if v >= 4:
                    emit_l2(v - 4)
                if v < npairs:
                    emit_msn(v)
                if 2 <= v < npairs + 2:
                    emit_gt(v - 2)
                if v < npairs:
                    emit_scores(v)
                    emit_gate(v)
                if 3 <= v < npairs + 3:
                    emit_l1(v - 3)
                if 2 <= v < npairs + 2:
                    emit_agg(v - 2)

    nc.compile()
    return nc


def _np_inputs_for_core(inputs, core, bpc=BPC):
    import ml_dtypes

    bf = ml_dtypes.bfloat16
    obs = np.asarray(inputs["obs_all"], np.float32)
    lo = core * bpc
    obsT = np.ascontiguousarray(
        obs[lo:lo + bpc].transpose(2, 0, 1)).astype(bf)

    W1 = np.asarray(inputs["W1"], np.float32)
    W2 = np.asarray(inputs["W2"], np.float32)
    b2 = np.asarray(inputs["b2"], np.float32)
    Wc = np.asarray(inputs["Wc"], np.float32)
    bc = np.asarray(inputs["bc"], np.float32)
    Wd = np.asarray(inputs["Wd"], np.float32)
    bd = np.asarray(inputs["bd"], np.float32)
    Wr1 = np.asarray(inputs["Wr1"], np.float32)
    br1 = np.asarray(inputs["br1"], np.float32)

    Wf = (W2 @ Wc) @ Wd                              # [H1, MSG]
    bf_vec = (b2 @ Wc) @ Wd + bc @ Wd + bd           # [MSG]
    Wr1c = Wr1[D:D + MSG]                            # [MSG, H2]
    # ones-row carries the folded message bias AND br1
    Wr1c_aug = np.vstack([Wr1c, (bf_vec @ Wr1c + br1)[None, :]])

    return {
        "obsT": obsT,
        "W1": W1.astype(bf),
        "Wf": Wf.astype(bf),
        "Wbil": np.asarray(inputs["Wbil"], np.float32).astype(bf),
        "Wr1a": Wr1[0:D].astype(bf),
        "Wr1c": np.ascontiguousarray(Wr1c_aug).astype(bf),
        "Wr2": np.asarray(inputs["Wr2"], np.float32).astype(bf),
        "b1": np.asarray(inputs["b1"], np.float32),
    }


def _finish(outT, br2):
    # outT: [D, bpc, N] bf16 -> [bpc, N, D] f32 + br2
    return outT.astype(np.float32).transpose(1, 2, 0) + br2[None, None, :]


def kernel(**inputs):
    from concourse.bass_utils import run_bass_kernel_spmd

    if "prog" not in _CACHE:
        _CACHE["prog"] = build_program(BPC)
    nc = _CACHE["prog"]

    br2 = np.asarray(inputs["br2"], np.float32)
    core_ids = list(range(NCORES))
    in_maps = [_np_inputs_for_core(inputs, c) for c in core_ids]
    res = run_bass_kernel_spmd(nc, in_maps, core_ids)
    out = np.concatenate(
        [_finish(np.asarray(res.results[c]["out"]), br2)
         for c in core_ids], axis=0)
    return out.astype(np.float32)


# revision 10
# speedup vs baseline: 2.2884x; 2.2884x over previous
"""Trainium2 Bass kernel for nn_BandwidthConstrainedComm.

GNN message passing: per batch element, N=256 agents each generate a
message (MLP -> compress -> decompress), compute pairwise bilinear
relevance scores, pick top-K=8 senders (softmax gated), aggregate their
messages, and run a receiver MLP over [obs, agg].

Sharding: pure data parallel over batch B=128 -> 16 per core x 8 cores.

Design notes (v2 - 5-stage software pipeline):
  - all inputs pre-cast to bf16 on the host; obs staged as [D, bpc, N]
    so every DMA line is a dense 1KB segment.
  - W2@Wc@Wd fused into one [H1, MSG] matrix on the host.
  - message bias bf = b2@Wc@Wd + bc@Wd + bd folded through the
    aggregation (gates sum to 1) into the receiver matmul via a
    ones-row in aggT and a host-precomputed (bf@Wr1c + br1) row
    appended to Wr1c -- br1 rides along for free, so the receiver
    relu needs no bias and runs as ONE [128,1024] activation.
  - br2 added on the host after gathering.
  - top-8 via DVE Max8 over exp'd scores; U = (E >= t8)*E via
    scalar_tensor_tensor on GPSIMD (idle engine), den = sum(top8) via
    gpsimd tensor_reduce, diag(1/den) built on gpsimd from a persistent
    identity. Only Max8 + one tiny reciprocal stay on DVE.
  - gate transpose+normalize in one regular matmul per 128-chunk:
    Gt = U.T @ diag(1/den).
  - per-pair work is split into FIVE pipeline stages emitted across
    iterations (pre(k) | gate(k) | Gt+aggT(k-2) | l1(k-3) | l2+out(k-4))
    so every cross-engine dependency has >= half an iteration of slack
    and the in-order tensor queue never head-of-line blocks.
  - engine load balance per pair (~5us each): PE all matmuls; scalar
    relu_h, out-cast, exp x2, relu_r, aggT-copy; vector tmpT-copy,
    msgs-copy, Gt-casts, Max8 x4, recip; gpsimd STT/den/diag x4.
  - PSUM: 4 tags / 14KB of 16KB: g[2KB x1]=hT->msn, a[4KB x1]=tmp->r,
    b[4KB x1]=o->s, c[2KB x2]=Gt0,Gt1,aggT (+warmup).
  - output written as bf16 [D, bpc, N] (dense lines), un-transposed and
    f32-cast on the host.
"""

import sys

sys.path.insert(0, "/opt/trn_rl_repo")

import numpy as np

# problem dims (hardcoded per contract)
B, N, D = 128, 256, 256
MSG, CD, K = 64, 32, 8
H1, H2 = 128, 256
NCORES = 8
BPC = B // NCORES  # batches per core

_CACHE = {}


def build_program(bpc=BPC, passes=1):
    import concourse.bacc as bacc
    import concourse.mybir as mybir
    import concourse.tile as tile
    from concourse.masks import make_identity
    from contextlib import ExitStack

    dt = mybir.dt
    f32, bf16 = dt.float32, dt.bfloat16
    AF = mybir.ActivationFunctionType
    OP = mybir.AluOpType

    assert bpc % 2 == 0
    npairs = bpc // 2

    nc = bacc.Bacc("TRN2", target_bir_lowering=False, debug=False,
                   num_devices=NCORES)

    obsT_d = nc.dram_tensor("obsT", [D, bpc, N], bf16, kind="ExternalInput")
    W1_d = nc.dram_tensor("W1", [D, H1], bf16, kind="ExternalInput")
    Wf_d = nc.dram_tensor("Wf", [H1, MSG], bf16, kind="ExternalInput")
    Wbil_d = nc.dram_tensor("Wbil", [D, D], bf16, kind="ExternalInput")
    Wr1a_d = nc.dram_tensor("Wr1a", [D, H2], bf16, kind="ExternalInput")
    Wr1c_d = nc.dram_tensor("Wr1c", [MSG + 1, H2], bf16,
                            kind="ExternalInput")
    Wr2_d = nc.dram_tensor("Wr2", [H2, D], bf16, kind="ExternalInput")
    b1_d = nc.dram_tensor("b1", [H1], f32, kind="ExternalInput")
    out_d = nc.dram_tensor("out", [D, bpc, N], bf16, kind="ExternalOutput")

    with tile.TileContext(nc) as tc, ExitStack() as ctx:
        wp = ctx.enter_context(tc.tile_pool(name="wp", bufs=1))
        dp = ctx.enter_context(tc.tile_pool(name="dp", bufs=5))
        sp = ctx.enter_context(tc.tile_pool(name="sp", bufs=3))
        pp = ctx.enter_context(tc.tile_pool(name="pp", bufs=1, space="PSUM"))

        # ---------------- one-time setup ----------------
        # warmup burst: dense PE work on a junk tile during the initial
        # DMA latency so the HAM clock-gate is at 8/8 when the first
        # real matmuls land.
        junk = wp.tile([128, 128], bf16, name="junk")
        nc.vector.memset(junk[:], 0.25)
        warm_ps = pp.tile([128, 128], f32, tag="c", bufs=2)
        for _ in range(16):
            nc.tensor.matmul(warm_ps[:], junk[:], junk[:],
                             start=True, stop=True)
        warm_sink = wp.tile([1, 8], f32, name="warm_sink")
        nc.vector.tensor_copy(warm_sink[:], warm_ps[0:1, 0:8])

        ident = wp.tile([128, 128], f32)
        make_identity(nc, ident[:])
        ident_b = wp.tile([128, 128], bf16)
        nc.vector.tensor_copy(ident_b[:], ident[:])

        def loadw(dram_ap, shape, name, eng=nc.scalar):
            t = wp.tile(shape, bf16, name=name)
            eng.dma_start(t[:], dram_ap)
            return t

        W1_r0 = loadw(W1_d[0:128, :], [128, H1], "W1a")
        W1_r1 = loadw(W1_d[128:256, :], [128, H1], "W1b", nc.gpsimd)
        Wf_b = loadw(Wf_d[:], [H1, MSG], "Wf")
        Wb_r0 = loadw(Wbil_d[0:128, :], [128, D], "Wba", nc.gpsimd)
        Wb_r1 = loadw(Wbil_d[128:256, :], [128, D], "Wbb")
        Wr1_r0 = loadw(Wr1a_d[0:128, :], [128, H2], "Wr1a", nc.gpsimd)
        Wr1_r1 = loadw(Wr1a_d[128:256, :], [128, H2], "Wr1b")
        Wr1c_b = loadw(Wr1c_d[:], [MSG + 1, H2], "Wr1c", nc.gpsimd)
        Wr2_r0 = loadw(Wr2_d[0:128, :], [128, D], "Wr2a")
        Wr2_r1 = loadw(Wr2_d[128:256, :], [128, D], "Wr2b", nc.gpsimd)

        b1_sb = wp.tile([H1, 1], f32, name="b1s")
        nc.scalar.dma_start(
            b1_sb[:], b1_d[:].rearrange("(p o) -> p o", o=1))

        # persistent aggT tiles with a constant ones-row (row MSG) for
        # the folded message bias (+ br1)
        aggT_tiles = []
        for i in range(2):
            t = wp.tile([MSG + 1, 2, N], bf16, name=f"aggTp{i}")
            nc.vector.memset(t[MSG:MSG + 1, :, :], 1.0)
            aggT_tiles.append(t)

        # ---------------- pipeline stages ----------------
        state = {}

        def emit_od(p):
            od_b = []
            for dc in range(2):
                ob = dp.tile([128, 2, N], bf16, name=f"od{dc}",
                             tag=f"od{dc}", bufs=5)
                nc.sync.dma_start(
                    ob[:], obsT_d[128 * dc:128 * (dc + 1),
                                  2 * p:2 * p + 2, :])
                od_b.append(ob[:].rearrange("d b n -> d (b n)"))
            state[("od", p)] = od_b

        def emit_ht(p):
            od_b = state[("od", p)]
            hT_ps = pp.tile([H1, 2 * N], f32, tag="g", bufs=1)
            nc.tensor.matmul(hT_ps[:], W1_r0[:], od_b[0],
                             start=True, stop=False)
            nc.tensor.matmul(hT_ps[:], W1_r1[:], od_b[1],
                             start=False, stop=True)
            hT_b = sp.tile([H1, 2 * N], bf16, name="hT_b", tag="hT",
                           bufs=2)
            nc.scalar.activation(hT_b[:], hT_ps[:], AF.Relu, bias=b1_sb[:])
            state[("hT", p)] = hT_b

        def emit_tmp(p):
            od_b = state[("od", p)]
            tmp_ps = pp.tile([128, 2, 2 * N], f32, tag="a", bufs=1)
            for ec in range(2):
                nc.tensor.matmul(tmp_ps[:, ec, :],
                                 Wb_r0[:, 128 * ec:128 * (ec + 1)],
                                 od_b[0], start=True, stop=False)
                nc.tensor.matmul(tmp_ps[:, ec, :],
                                 Wb_r1[:, 128 * ec:128 * (ec + 1)],
                                 od_b[1], start=False, stop=True)
            tmpT_r = sp.tile([128, 2, 2 * N], bf16, name="tmpT_r",
                             tag="tmpT", bufs=2)
            nc.vector.tensor_copy(
                tmpT_r[:].rearrange("e c f -> e (c f)"),
                tmp_ps[:].rearrange("e c f -> e (c f)"))
            state[("tmpT", p)] = tmpT_r

        def emit_msn(p):
            hT_b = state.pop(("hT", p))
            msn_ps = pp.tile([128, 4, MSG], f32, tag="g", bufs=1,
                             name="msn_ps")
            for q in range(4):
                nc.tensor.matmul(msn_ps[:, q, :],
                                 hT_b[:, 128 * q:128 * (q + 1)],
                                 Wf_b[:], start=True, stop=True)
            msgs_b = sp.tile([128, 4, MSG], bf16, name="msgs_b",
                             tag="msgs", bufs=3)
            nc.vector.tensor_copy(msgs_b[:], msn_ps[:])
            state[("msgs", p)] = msgs_b

        def emit_scores(p):
            od_b = state[("od", p)]
            tmpT_r = state.pop(("tmpT", p))
            s_ps = pp.tile([128, 2, 2, N], f32, tag="b", bufs=1,
                           name="s_ps")
            for bi in range(2):
                boff = bi * N
                for ic in range(2):
                    ioff = boff + 128 * ic
                    nc.tensor.matmul(s_ps[:, bi, ic, :],
                                     tmpT_r[:, 0, ioff:ioff + 128],
                                     od_b[0][:, boff:boff + N],
                                     start=True, stop=False)
                    nc.tensor.matmul(s_ps[:, bi, ic, :],
                                     tmpT_r[:, 1, ioff:ioff + 128],
                                     od_b[1][:, boff:boff + N],
                                     start=False, stop=True)
            state[("s_ps", p)] = s_ps

        def emit_exp(p):
            s_ps = state.pop(("s_ps", p))
            E = sp.tile([128, 4, N], bf16, name="E", tag="E", bufs=4)
            nc.scalar.activation(
                E[:].rearrange("p c f -> p (c f)"),
                s_ps[:].rearrange("p b c f -> p (b c f)"), AF.Exp)
            state[("E", p)] = E

        def emit_gate(p):
            # top-8, den, rden, msk=(E>=t8)*rden on DVE (tensor_scalar
            # runs the 2x single-src path); U = msk*E on GPSIMD.
            E = state.pop(("E", p))
            top8 = sp.tile([128, 4, 8], f32, name="top8", tag="top8",
                           bufs=3)
            for c in range(4):
                nc.vector.max(out=top8[:, c, :], in_=E[:, c, :])
            den = sp.tile([128, 4], f32, name="den", tag="den", bufs=3)
            nc.vector.tensor_reduce(
                out=den[:], in_=top8[:], axis=mybir.AxisListType.X,
                op=OP.add)
            rden = sp.tile([128, 4], f32, name="rden", tag="rden", bufs=3)
            nc.vector.reciprocal(rden[:], den[:])
            Us = []
            for bi in range(2):
                U = sp.tile([128, 2, N], bf16, name="U", tag=f"U{bi}",
                            bufs=3)
                for ic in range(2):
                    c = 2 * bi + ic
                    msk = sp.tile([128, N], bf16, name="msk", tag="msk",
                                  bufs=4)
                    nc.vector.tensor_scalar(
                        out=msk[:], in0=E[:, c, :],
                        scalar1=top8[:, c, 7:8],
                        scalar2=rden[:, c:c + 1],
                        op0=OP.is_ge, op1=OP.mult)
                    nc.gpsimd.tensor_tensor(
                        out=U[:, ic, :], in0=msk[:],
                        in1=E[:, c, :], op=OP.mult)
                Us.append(U)
            state[("gate", p)] = Us

        def emit_gt(p):
            Us = state.pop(("gate", p))
            Gt_bs = []
            for bi in range(2):
                U = Us[bi]
                Gt_ps = pp.tile([128, 2, N], f32, tag="c", bufs=2,
                                name="Gt_ps")
                for ic in range(2):
                    for jc in range(2):
                        nc.tensor.matmul(
                            Gt_ps[:, jc, 128 * ic:128 * (ic + 1)],
                            U[:, ic, 128 * jc:128 * (jc + 1)],
                            ident_b[:], start=True, stop=True)
                Gt_b = sp.tile([128, 2, N], bf16, name="Gt_b", tag="Gt",
                               bufs=4)
                nc.scalar.activation(
                    Gt_b[:].rearrange("p c f -> p (c f)"),
                    Gt_ps[:].rearrange("p c f -> p (c f)"), AF.Copy)
                Gt_bs.append(Gt_b)
            state[("Gt", p)] = Gt_bs

        def emit_agg(p):
            Gt_bs = state.pop(("Gt", p))
            msgs_b = state.pop(("msgs", p))
            aggT_ps = pp.tile([MSG, 2, N], f32, tag="c", bufs=2,
                              name="aggT_ps")
            for bi in range(2):
                nc.tensor.matmul(aggT_ps[:, bi, :],
                                 msgs_b[:, 2 * bi, :], Gt_bs[bi][:, 0, :],
                                 start=True, stop=False)
                nc.tensor.matmul(aggT_ps[:, bi, :],
                                 msgs_b[:, 2 * bi + 1, :],
                                 Gt_bs[bi][:, 1, :],
                                 start=False, stop=True)
            aggT_r = aggT_tiles[p % 2]
            nc.vector.tensor_copy(
                aggT_r[0:MSG, :, :].rearrange("m b n -> m (b n)"),
                aggT_ps[:].rearrange("m b n -> m (b n)"))

        def emit_l1(p):
            od_b = state[("od", p)]
            aggT_r = aggT_tiles[p % 2]
            r_ps = pp.tile([128, 2, 2 * N], f32, tag="a", bufs=1,
                           name="r_ps")
            aggT_ap = aggT_r[:].rearrange("m b n -> m (b n)")
            for mi in range(2):
                ms = 128 * mi
                nc.tensor.matmul(r_ps[:, mi, :], Wr1_r0[:, ms:ms + 128],
                                 od_b[0], start=True, stop=False)
                nc.tensor.matmul(r_ps[:, mi, :], Wr1_r1[:, ms:ms + 128],
                                 od_b[1], start=False, stop=False)
                nc.tensor.matmul(r_ps[:, mi, :], Wr1c_b[:, ms:ms + 128],
                                 aggT_ap, start=False, stop=True)
            state[("r_ps", p)] = r_ps

        def emit_relu_r(p):
            r_ps = state.pop(("r_ps", p))
            rT = sp.tile([128, 2, 2 * N], bf16, name="rT", tag="rT",
                         bufs=3)
            nc.scalar.activation(
                rT[:].rearrange("h c f -> h (c f)"),
                r_ps[:].rearrange("h c f -> h (c f)"), AF.Relu)
            state[("rT", p)] = rT

        def emit_l2(p):
            rT = state.pop(("rT", p))
            state.pop(("od", p))
            o_ps = pp.tile([128, 2, 2 * N], f32, tag="b", bufs=1,
                           name="o_ps")
            for dc in range(2):
                ds = 128 * dc
                nc.tensor.matmul(o_ps[:, dc, :], Wr2_r0[:, ds:ds + 128],
                                 rT[:, 0, :], start=True, stop=False)
                nc.tensor.matmul(o_ps[:, dc, :], Wr2_r1[:, ds:ds + 128],
                                 rT[:, 1, :], start=False, stop=True)
            o_sb = sp.tile([128, 2, 2, N], bf16, name="o_sb", tag="o_sb",
                           bufs=2)
            nc.scalar.activation(
                o_sb[:].rearrange("d c b n -> d (c b n)"),
                o_ps[:].rearrange("d c f -> d (c f)"), AF.Copy)
            b0 = 2 * p
            for dc in range(2):
                nc.sync.dma_start(
                    out_d[128 * dc:128 * (dc + 1), b0:b0 + 2, :],
                    o_sb[:, dc])

        # ---------------- main pipeline loop ----------------
        # Per-iteration emission order fixes each engine's queue order:
        #   PE:     hT(v) tmp(v) l2(v-4) msn(v) Gt(v-2) s(v) l1(v-3)
        #           aggT(v-2)
        #   scalar: relu_h(v) out(v-4) Gt-cast(v-2) relu_r(v-3) exp(v)
        #   vector: tmpT(v) msn-cp(v) aggT-cp(v-2) Max8/den/msk(v)
        #   gpsimd: U-mult(v) x4
        for _ in range(passes):
            emit_od(0)
            for v in range(npairs + 4):
                if v + 1 < npairs:
                    emit_od(v + 1)
                if v < npairs:
                    emit_ht(v)
                    emit_tmp(v)
                if v >= 4:
                    emit_l2(v - 4)
                if v < npairs:
                    emit_msn(v)
                if 2 <= v < npairs + 2:
                    emit_gt(v - 2)
                if v < npairs:
                    emit_scores(v)
                if 3 <= v < npairs + 3:
                    emit_l1(v - 3)
                    emit_relu_r(v - 3)
                if v < npairs:
                    emit_exp(v)
                if 2 <= v < npairs + 2:
                    emit_agg(v - 2)
                if v < npairs:
                    emit_gate(v)

    nc.compile()
    return nc


def _np_inputs_for_core(inputs, core, bpc=BPC):
    import ml_dtypes

    bf = ml_dtypes.bfloat16
    obs = np.asarray(inputs["obs_all"], np.float32)
    lo = core * bpc
    obsT = np.ascontiguousarray(
        obs[lo:lo + bpc].transpose(2, 0, 1)).astype(bf)

    W1 = np.asarray(inputs["W1"], np.float32)
    W2 = np.asarray(inputs["W2"], np.float32)
    b2 = np.asarray(inputs["b2"], np.float32)
    Wc = np.asarray(inputs["Wc"], np.float32)
    bc = np.asarray(inputs["bc"], np.float32)
    Wd = np.asarray(inputs["Wd"], np.float32)
    bd = np.asarray(inputs["bd"], np.float32)
    Wr1 = np.asarray(inputs["Wr1"], np.float32)
    br1 = np.asarray(inputs["br1"], np.float32)

    Wf = (W2 @ Wc) @ Wd                              # [H1, MSG]
    bf_vec = (b2 @ Wc) @ Wd + bc @ Wd + bd           # [MSG]
    Wr1c = Wr1[D:D + MSG]                            # [MSG, H2]
    # ones-row carries the folded message bias AND br1
    Wr1c_aug = np.vstack([Wr1c, (bf_vec @ Wr1c + br1)[None, :]])

    return {
        "obsT": obsT,
        "W1": W1.astype(bf),
        "Wf": Wf.astype(bf),
        "Wbil": np.asarray(inputs["Wbil"], np.float32).astype(bf),
        "Wr1a": Wr1[0:D].astype(bf),
        "Wr1c": np.ascontiguousarray(Wr1c_aug).astype(bf),
        "Wr2": np.asarray(inputs["Wr2"], np.float32).astype(bf),
        "b1": np.asarray(inputs["b1"], np.float32),
    }


def _finish(outT, br2):
    # outT: [D, bpc, N] bf16 -> [bpc, N, D] f32 + br2
    return outT.astype(np.float32).transpose(1, 2, 0) + br2[None, None, :]


def kernel(**inputs):
    from concourse.bass_utils import run_bass_kernel_spmd

    if "prog" not in _CACHE:
        _CACHE["prog"] = build_program(BPC)
    nc = _CACHE["prog"]

    br2 = np.asarray(inputs["br2"], np.float32)
    core_ids = list(range(NCORES))
    in_maps = [_np_inputs_for_core(inputs, c) for c in core_ids]
    res = run_bass_kernel_spmd(nc, in_maps, core_ids)
    out = np.concatenate(
        [_finish(np.asarray(res.results[c]["out"]), br2)
         for c in core_ids], axis=0)
    return out.astype(np.float32)


# revision 11
# speedup vs baseline: 2.3326x; 1.0193x over previous
"""Trainium2 Bass kernel for nn_BandwidthConstrainedComm.

GNN message passing: per batch element, N=256 agents each generate a
message (MLP -> compress -> decompress), compute pairwise bilinear
relevance scores, pick top-K=8 senders (softmax gated), aggregate their
messages, and run a receiver MLP over [obs, agg].

Sharding: pure data parallel over batch B=128 -> 16 per core x 8 cores.

Design notes (v2 - 5-stage software pipeline):
  - all inputs pre-cast to bf16 on the host; obs staged as [D, bpc, N]
    so every DMA line is a dense 1KB segment.
  - W2@Wc@Wd fused into one [H1, MSG] matrix on the host.
  - message bias bf = b2@Wc@Wd + bc@Wd + bd folded through the
    aggregation (gates sum to 1) into the receiver matmul via a
    ones-row in aggT and a host-precomputed (bf@Wr1c + br1) row
    appended to Wr1c -- br1 rides along for free, so the receiver
    relu needs no bias and runs as ONE [128,1024] activation.
  - br2 added on the host after gathering.
  - top-8 via DVE Max8 over exp'd scores; U = (E >= t8)*E via
    scalar_tensor_tensor on GPSIMD (idle engine), den = sum(top8) via
    gpsimd tensor_reduce, diag(1/den) built on gpsimd from a persistent
    identity. Only Max8 + one tiny reciprocal stay on DVE.
  - gate transpose+normalize in one regular matmul per 128-chunk:
    Gt = U.T @ diag(1/den).
  - per-pair work is split into FIVE pipeline stages emitted across
    iterations (pre(k) | gate(k) | Gt+aggT(k-2) | l1(k-3) | l2+out(k-4))
    so every cross-engine dependency has >= half an iteration of slack
    and the in-order tensor queue never head-of-line blocks.
  - engine load balance per pair (~5us each): PE all matmuls; scalar
    relu_h, out-cast, exp x2, relu_r, aggT-copy; vector tmpT-copy,
    msgs-copy, Gt-casts, Max8 x4, recip; gpsimd STT/den/diag x4.
  - PSUM: 4 tags / 14KB of 16KB: g[2KB x1]=hT->msn, a[4KB x1]=tmp->r,
    b[4KB x1]=o->s, c[2KB x2]=Gt0,Gt1,aggT (+warmup).
  - output written as bf16 [D, bpc, N] (dense lines), un-transposed and
    f32-cast on the host.
"""

import sys

sys.path.insert(0, "/opt/trn_rl_repo")

import numpy as np

# problem dims (hardcoded per contract)
B, N, D = 128, 256, 256
MSG, CD, K = 64, 32, 8
H1, H2 = 128, 256
NCORES = 8
BPC = B // NCORES  # batches per core

_CACHE = {}


def build_program(bpc=BPC, passes=1):
    import concourse.bacc as bacc
    import concourse.mybir as mybir
    import concourse.tile as tile
    from concourse.masks import make_identity
    from contextlib import ExitStack

    dt = mybir.dt
    f32, bf16 = dt.float32, dt.bfloat16
    AF = mybir.ActivationFunctionType
    OP = mybir.AluOpType

    assert bpc % 2 == 0
    npairs = bpc // 2

    nc = bacc.Bacc("TRN2", target_bir_lowering=False, debug=False,
                   num_devices=NCORES)

    obsT_d = nc.dram_tensor("obsT", [D, bpc, N], bf16, kind="ExternalInput")
    W1_d = nc.dram_tensor("W1", [D, H1], bf16, kind="ExternalInput")
    Wf_d = nc.dram_tensor("Wf", [H1, MSG], bf16, kind="ExternalInput")
    Wbil_d = nc.dram_tensor("Wbil", [D, D], bf16, kind="ExternalInput")
    Wr1a_d = nc.dram_tensor("Wr1a", [D, H2], bf16, kind="ExternalInput")
    Wr1c_d = nc.dram_tensor("Wr1c", [MSG + 1, H2], bf16,
                            kind="ExternalInput")
    Wr2_d = nc.dram_tensor("Wr2", [H2, D], bf16, kind="ExternalInput")
    b1_d = nc.dram_tensor("b1", [H1], f32, kind="ExternalInput")
    out_d = nc.dram_tensor("out", [D, bpc, N], bf16, kind="ExternalOutput")

    with tile.TileContext(nc) as tc, ExitStack() as ctx:
        wp = ctx.enter_context(tc.tile_pool(name="wp", bufs=1))
        dp = ctx.enter_context(tc.tile_pool(name="dp", bufs=5))
        sp = ctx.enter_context(tc.tile_pool(name="sp", bufs=3))
        pp = ctx.enter_context(tc.tile_pool(name="pp", bufs=1, space="PSUM"))

        # ---------------- one-time setup ----------------
        # warmup burst: dense PE work on a junk tile during the initial
        # DMA latency so the HAM clock-gate is at 8/8 when the first
        # real matmuls land.
        junk = wp.tile([128, 128], bf16, name="junk")
        nc.vector.memset(junk[:], 0.25)
        warm_ps = pp.tile([128, 128], f32, tag="c", bufs=2)
        for _ in range(16):
            nc.tensor.matmul(warm_ps[:], junk[:], junk[:],
                             start=True, stop=True)
        warm_sink = wp.tile([1, 8], f32, name="warm_sink")
        nc.vector.tensor_copy(warm_sink[:], warm_ps[0:1, 0:8])

        ident = wp.tile([128, 128], f32)
        make_identity(nc, ident[:])
        ident_b = wp.tile([128, 128], bf16)
        nc.vector.tensor_copy(ident_b[:], ident[:])

        def loadw(dram_ap, shape, name, eng=nc.scalar):
            t = wp.tile(shape, bf16, name=name)
            eng.dma_start(t[:], dram_ap)
            return t

        W1_r0 = loadw(W1_d[0:128, :], [128, H1], "W1a")
        W1_r1 = loadw(W1_d[128:256, :], [128, H1], "W1b", nc.gpsimd)
        Wf_b = loadw(Wf_d[:], [H1, MSG], "Wf")
        Wb_r0 = loadw(Wbil_d[0:128, :], [128, D], "Wba", nc.gpsimd)
        Wb_r1 = loadw(Wbil_d[128:256, :], [128, D], "Wbb")
        Wr1_r0 = loadw(Wr1a_d[0:128, :], [128, H2], "Wr1a", nc.gpsimd)
        Wr1_r1 = loadw(Wr1a_d[128:256, :], [128, H2], "Wr1b")
        Wr1c_b = loadw(Wr1c_d[:], [MSG + 1, H2], "Wr1c", nc.gpsimd)
        Wr2_r0 = loadw(Wr2_d[0:128, :], [128, D], "Wr2a")
        Wr2_r1 = loadw(Wr2_d[128:256, :], [128, D], "Wr2b", nc.gpsimd)

        b1_sb = wp.tile([H1, 1], f32, name="b1s")
        nc.scalar.dma_start(
            b1_sb[:], b1_d[:].rearrange("(p o) -> p o", o=1))

        # persistent aggT tiles with a constant ones-row (row MSG) for
        # the folded message bias (+ br1)
        aggT_tiles = []
        for i in range(2):
            t = wp.tile([MSG + 1, 2, N], bf16, name=f"aggTp{i}")
            nc.vector.memset(t[MSG:MSG + 1, :, :], 1.0)
            aggT_tiles.append(t)

        # ---------------- pipeline stages ----------------
        state = {}

        def emit_od(p):
            od_b = []
            for dc in range(2):
                ob = dp.tile([128, 2, N], bf16, name=f"od{dc}",
                             tag=f"od{dc}", bufs=5)
                nc.sync.dma_start(
                    ob[:], obsT_d[128 * dc:128 * (dc + 1),
                                  2 * p:2 * p + 2, :])
                od_b.append(ob[:].rearrange("d b n -> d (b n)"))
            state[("od", p)] = od_b

        def emit_ht(p):
            od_b = state[("od", p)]
            hT_ps = pp.tile([H1, 2 * N], f32, tag="g", bufs=1)
            nc.tensor.matmul(hT_ps[:], W1_r0[:], od_b[0],
                             start=True, stop=False)
            nc.tensor.matmul(hT_ps[:], W1_r1[:], od_b[1],
                             start=False, stop=True)
            hT_b = sp.tile([H1, 2 * N], bf16, name="hT_b", tag="hT",
                           bufs=2)
            nc.scalar.activation(hT_b[:], hT_ps[:], AF.Relu, bias=b1_sb[:])
            state[("hT", p)] = hT_b

        def emit_tmp(p):
            od_b = state[("od", p)]
            tmp_ps = pp.tile([128, 2, 2 * N], f32, tag="a", bufs=1)
            for ec in range(2):
                nc.tensor.matmul(tmp_ps[:, ec, :],
                                 Wb_r0[:, 128 * ec:128 * (ec + 1)],
                                 od_b[0], start=True, stop=False)
                nc.tensor.matmul(tmp_ps[:, ec, :],
                                 Wb_r1[:, 128 * ec:128 * (ec + 1)],
                                 od_b[1], start=False, stop=True)
            tmpT_r = sp.tile([128, 2, 2 * N], bf16, name="tmpT_r",
                             tag="tmpT", bufs=2)
            nc.vector.tensor_copy(
                tmpT_r[:].rearrange("e c f -> e (c f)"),
                tmp_ps[:].rearrange("e c f -> e (c f)"))
            state[("tmpT", p)] = tmpT_r

        def emit_msn(p):
            hT_b = state.pop(("hT", p))
            msn_ps = pp.tile([128, 4, MSG], f32, tag="g", bufs=1,
                             name="msn_ps")
            for q in range(4):
                nc.tensor.matmul(msn_ps[:, q, :],
                                 hT_b[:, 128 * q:128 * (q + 1)],
                                 Wf_b[:], start=True, stop=True)
            msgs_b = sp.tile([128, 4, MSG], bf16, name="msgs_b",
                             tag="msgs", bufs=3)
            nc.vector.tensor_copy(msgs_b[:], msn_ps[:])
            state[("msgs", p)] = msgs_b

        def emit_scores(p):
            od_b = state[("od", p)]
            tmpT_r = state.pop(("tmpT", p))
            s_ps = pp.tile([128, 2, 2, N], f32, tag="b", bufs=1,
                           name="s_ps")
            for bi in range(2):
                boff = bi * N
                for ic in range(2):
                    ioff = boff + 128 * ic
                    nc.tensor.matmul(s_ps[:, bi, ic, :],
                                     tmpT_r[:, 0, ioff:ioff + 128],
                                     od_b[0][:, boff:boff + N],
                                     start=True, stop=False)
                    nc.tensor.matmul(s_ps[:, bi, ic, :],
                                     tmpT_r[:, 1, ioff:ioff + 128],
                                     od_b[1][:, boff:boff + N],
                                     start=False, stop=True)
            state[("s_ps", p)] = s_ps

        def emit_exp(p):
            s_ps = state.pop(("s_ps", p))
            E = sp.tile([128, 4, N], bf16, name="E", tag="E", bufs=4)
            nc.scalar.activation(
                E[:].rearrange("p c f -> p (c f)"),
                s_ps[:].rearrange("p b c f -> p (b c f)"), AF.Exp)
            state[("E", p)] = E

        def emit_gate(p):
            # top-8, den, rden, msk=(E>=t8)*rden on DVE (tensor_scalar
            # runs the 2x single-src path); U = msk*E on GPSIMD.
            E = state.pop(("E", p))
            top8 = sp.tile([128, 4, 8], f32, name="top8", tag="top8",
                           bufs=3)
            for c in range(4):
                nc.vector.max(out=top8[:, c, :], in_=E[:, c, :])
            den = sp.tile([128, 4], f32, name="den", tag="den", bufs=3)
            nc.vector.tensor_reduce(
                out=den[:], in_=top8[:], axis=mybir.AxisListType.X,
                op=OP.add)
            rden = sp.tile([128, 4], f32, name="rden", tag="rden", bufs=3)
            nc.vector.reciprocal(rden[:], den[:])
            msks = []
            for c in range(4):
                msk = sp.tile([128, N], bf16, name="msk", tag="msk",
                              bufs=6)
                nc.vector.tensor_scalar(
                    out=msk[:], in0=E[:, c, :],
                    scalar1=top8[:, c, 7:8],
                    scalar2=rden[:, c:c + 1],
                    op0=OP.is_ge, op1=OP.mult)
                msks.append(msk)
            Us = []
            for bi in range(2):
                U = sp.tile([128, 2, N], bf16, name="U", tag=f"U{bi}",
                            bufs=3)
                for ic in range(2):
                    c = 2 * bi + ic
                    nc.gpsimd.tensor_tensor(
                        out=U[:, ic, :], in0=msks[c],
                        in1=E[:, c, :], op=OP.mult)
                Us.append(U)
            state[("gate", p)] = Us

        def emit_gt(p):
            Us = state.pop(("gate", p))
            Gt_bs = []
            for bi in range(2):
                U = Us[bi]
                Gt_ps = pp.tile([128, 2, N], f32, tag="c", bufs=2,
                                name="Gt_ps")
                for ic in range(2):
                    for jc in range(2):
                        nc.tensor.matmul(
                            Gt_ps[:, jc, 128 * ic:128 * (ic + 1)],
                            U[:, ic, 128 * jc:128 * (jc + 1)],
                            ident_b[:], start=True, stop=True)
                Gt_b = sp.tile([128, 2, N], bf16, name="Gt_b", tag="Gt",
                               bufs=4)
                nc.scalar.activation(
                    Gt_b[:].rearrange("p c f -> p (c f)"),
                    Gt_ps[:].rearrange("p c f -> p (c f)"), AF.Copy)
                Gt_bs.append(Gt_b)
            state[("Gt", p)] = Gt_bs

        def emit_agg(p):
            Gt_bs = state.pop(("Gt", p))
            msgs_b = state.pop(("msgs", p))
            aggT_ps = pp.tile([MSG, 2, N], f32, tag="c", bufs=2,
                              name="aggT_ps")
            for bi in range(2):
                nc.tensor.matmul(aggT_ps[:, bi, :],
                                 msgs_b[:, 2 * bi, :], Gt_bs[bi][:, 0, :],
                                 start=True, stop=False)
                nc.tensor.matmul(aggT_ps[:, bi, :],
                                 msgs_b[:, 2 * bi + 1, :],
                                 Gt_bs[bi][:, 1, :],
                                 start=False, stop=True)
            state[("aggT_ps", p)] = aggT_ps

        def emit_agg_cp(p):
            aggT_ps = state.pop(("aggT_ps", p))
            aggT_r = aggT_tiles[p % 2]
            nc.vector.tensor_copy(
                aggT_r[0:MSG, :, :].rearrange("m b n -> m (b n)"),
                aggT_ps[:].rearrange("m b n -> m (b n)"))

        def emit_l1(p):
            od_b = state[("od", p)]
            aggT_r = aggT_tiles[p % 2]
            r_ps = pp.tile([128, 2, 2 * N], f32, tag="a", bufs=1,
                           name="r_ps")
            aggT_ap = aggT_r[:].rearrange("m b n -> m (b n)")
            for mi in range(2):
                ms = 128 * mi
                nc.tensor.matmul(r_ps[:, mi, :], Wr1_r0[:, ms:ms + 128],
                                 od_b[0], start=True, stop=False)
                nc.tensor.matmul(r_ps[:, mi, :], Wr1_r1[:, ms:ms + 128],
                                 od_b[1], start=False, stop=False)
                nc.tensor.matmul(r_ps[:, mi, :], Wr1c_b[:, ms:ms + 128],
                                 aggT_ap, start=False, stop=True)
            state[("r_ps", p)] = r_ps

        def emit_relu_r(p):
            r_ps = state.pop(("r_ps", p))
            rT = sp.tile([128, 2, 2 * N], bf16, name="rT", tag="rT",
                         bufs=3)
            nc.scalar.activation(
                rT[:].rearrange("h c f -> h (c f)"),
                r_ps[:].rearrange("h c f -> h (c f)"), AF.Relu)
            state[("rT", p)] = rT

        def emit_l2(p):
            rT = state.pop(("rT", p))
            state.pop(("od", p))
            o_ps = pp.tile([128, 2, 2 * N], f32, tag="b", bufs=1,
                           name="o_ps")
            for dc in range(2):
                ds = 128 * dc
                nc.tensor.matmul(o_ps[:, dc, :], Wr2_r0[:, ds:ds + 128],
                                 rT[:, 0, :], start=True, stop=False)
                nc.tensor.matmul(o_ps[:, dc, :], Wr2_r1[:, ds:ds + 128],
                                 rT[:, 1, :], start=False, stop=True)
            o_sb = sp.tile([128, 2, 2, N], bf16, name="o_sb", tag="o_sb",
                           bufs=2)
            nc.scalar.activation(
                o_sb[:].rearrange("d c b n -> d (c b n)"),
                o_ps[:].rearrange("d c f -> d (c f)"), AF.Copy)
            b0 = 2 * p
            for dc in range(2):
                nc.sync.dma_start(
                    out_d[128 * dc:128 * (dc + 1), b0:b0 + 2, :],
                    o_sb[:, dc])

        # ---------------- main pipeline loop ----------------
        # Per-iteration emission order fixes each engine's queue order:
        #   PE:     hT(v) tmp(v) l2(v-4) msn(v) Gt(v-2) s(v) l1(v-3)
        #           aggT(v-2)
        #   scalar: relu_h(v) out(v-4) Gt-cast(v-2) relu_r(v-3) exp(v)
        #   vector: tmpT(v) msn-cp(v) aggT-cp(v-2) Max8/den/msk(v)
        #   gpsimd: U-mult(v) x4
        for _ in range(passes):
            emit_od(0)
            for v in range(npairs + 4):
                if v + 1 < npairs:
                    emit_od(v + 1)
                if v < npairs:
                    emit_ht(v)
                    emit_tmp(v)
                if v >= 4:
                    emit_l2(v - 4)
                if v < npairs:
                    emit_msn(v)
                if 2 <= v < npairs + 2:
                    emit_gt(v - 2)
                if v < npairs:
                    emit_scores(v)
                if 3 <= v < npairs + 3:
                    emit_l1(v - 3)
                    emit_relu_r(v - 3)
                if v < npairs:
                    emit_exp(v)
                if 2 <= v < npairs + 2:
                    emit_agg(v - 2)
                if 1 <= v < npairs + 1:
                    emit_gate(v - 1)
                if 2 <= v < npairs + 2:
                    emit_agg_cp(v - 2)

    nc.compile()
    return nc


def _np_inputs_for_core(inputs, core, bpc=BPC):
    import ml_dtypes

    bf = ml_dtypes.bfloat16
    obs = np.asarray(inputs["obs_all"], np.float32)
    lo = core * bpc
    obsT = np.ascontiguousarray(
        obs[lo:lo + bpc].transpose(2, 0, 1)).astype(bf)

    W1 = np.asarray(inputs["W1"], np.float32)
    W2 = np.asarray(inputs["W2"], np.float32)
    b2 = np.asarray(inputs["b2"], np.float32)
    Wc = np.asarray(inputs["Wc"], np.float32)
    bc = np.asarray(inputs["bc"], np.float32)
    Wd = np.asarray(inputs["Wd"], np.float32)
    bd = np.asarray(inputs["bd"], np.float32)
    Wr1 = np.asarray(inputs["Wr1"], np.float32)
    br1 = np.asarray(inputs["br1"], np.float32)

    Wf = (W2 @ Wc) @ Wd                              # [H1, MSG]
    bf_vec = (b2 @ Wc) @ Wd + bc @ Wd + bd           # [MSG]
    Wr1c = Wr1[D:D + MSG]                            # [MSG, H2]
    # ones-row carries the folded message bias AND br1
    Wr1c_aug = np.vstack([Wr1c, (bf_vec @ Wr1c + br1)[None, :]])

    return {
        "obsT": obsT,
        "W1": W1.astype(bf),
        "Wf": Wf.astype(bf),
        "Wbil": np.asarray(inputs["Wbil"], np.float32).astype(bf),
        "Wr1a": Wr1[0:D].astype(bf),
        "Wr1c": np.ascontiguousarray(Wr1c_aug).astype(bf),
        "Wr2": np.asarray(inputs["Wr2"], np.float32).astype(bf),
        "b1": np.asarray(inputs["b1"], np.float32),
    }


def _finish(outT, br2):
    # outT: [D, bpc, N] bf16 -> [bpc, N, D] f32 + br2
    return outT.astype(np.float32).transpose(1, 2, 0) + br2[None, None, :]


def kernel(**inputs):
    from concourse.bass_utils import run_bass_kernel_spmd

    if "prog" not in _CACHE:
        _CACHE["prog"] = build_program(BPC)
    nc = _CACHE["prog"]

    br2 = np.asarray(inputs["br2"], np.float32)
    core_ids = list(range(NCORES))
    in_maps = [_np_inputs_for_core(inputs, c) for c in core_ids]
    res = run_bass_kernel_spmd(nc, in_maps, core_ids)
    out = np.concatenate(
        [_finish(np.asarray(res.results[c]["out"]), br2)
         for c in core_ids], axis=0)
    return out.astype(np.float32)


# revision 12
# speedup vs baseline: 2.4024x; 1.0299x over previous
"""Trainium2 Bass kernel for nn_BandwidthConstrainedComm.

GNN message passing: per batch element, N=256 agents each generate a
message (MLP -> compress -> decompress), compute pairwise bilinear
relevance scores, pick top-K=8 senders (softmax gated), aggregate their
messages, and run a receiver MLP over [obs, agg].

Sharding: pure data parallel over batch B=128 -> 16 per core x 8 cores.

Design notes (v4 - 5-stage pipeline + fp8 DoubleRow front-end):
  - obs uploaded twice: bf16 [d,2(dc),b,n] for the receiver MLP (fp8
    there breaks tolerance) and fp8-e4m3 [d_lo,2(d_hi),b,n] for the
    message/score path (verified: max-err unchanged, the bf16 receiver
    path dominates).
  - h-MLP, bilinear tmp and scores run as fp8 DoubleRow matmuls
    (K=256 per instruction, ~2x fewer PE instructions, 1.44x rate).
    tmpT is cast PSUM->fp8 so scores' stationary operand is fp8 too.
  - W2@Wc@Wd fused into one [H1, MSG] matrix on the host; message bias
    (+ br1) folded into the receiver matmul via a ones-row in aggT and
    an extra host-precomputed row in Wr1c; br2 added on the host.
  - top-8 via DVE Max8; den=sum(top8) one reduce; msk=(E>=t8)*rden on
    DVE (2-scalar tensor_scalar); U = msk*E on GPSIMD tensor_tensor
    (the only fast Pool op); gate transpose Gt = U.T @ I by PE.
  - 5-stage pipeline: pre(v) | gate(v-1) | Gt+agg(v-2) | l1(v-3) |
    l2+out(v-4), with per-engine queue orders chosen so every
    cross-engine dependency has >= half an iteration of slack.
  - engine budget per pair: PE ~4us, scalar relu_h/out/msn/Gt-cast/
    relu_r/exp ~5.5us, vector tmpT/Max8/den/recip/msk/aggT ~5.3us,
    gpsimd 4 mults ~2.7us.
  - PSUM 16KB/partition exactly: g[2K]=hT/msn, a[4K]=tmp/r, b[4K]=o/s,
    c[4K]=Gt, e[2K]=aggT.
  - output written as bf16 [D, bpc, N] in one DMA, un-transposed and
    f32-cast on the host.
"""

import sys

sys.path.insert(0, "/opt/trn_rl_repo")

import numpy as np

# problem dims (hardcoded per contract)
B, N, D = 128, 256, 256
MSG, CD, K = 64, 32, 8
H1, H2 = 128, 256
NCORES = 8
BPC = B // NCORES  # batches per core

_CACHE = {}


def build_program(bpc=BPC, passes=1):
    import concourse.bacc as bacc
    import concourse.mybir as mybir
    import concourse.tile as tile
    from concourse.masks import make_identity
    from contextlib import ExitStack

    dt = mybir.dt
    f32, bf16, f8 = dt.float32, dt.bfloat16, dt.float8e4
    AF = mybir.ActivationFunctionType
    OP = mybir.AluOpType
    DR = mybir.MatmulPerfMode.DoubleRow

    assert bpc % 2 == 0
    npairs = bpc // 2

    nc = bacc.Bacc("TRN2", target_bir_lowering=False, debug=False,
                   num_devices=NCORES)

    obsT_d = nc.dram_tensor("obsT", [D, bpc, N], bf16, kind="ExternalInput")
    obs8_d = nc.dram_tensor("obs8", [128, 2, bpc, N], f8,
                            kind="ExternalInput")
    W1_d = nc.dram_tensor("W1", [128, 2, H1], f8, kind="ExternalInput")
    Wf_d = nc.dram_tensor("Wf", [H1, MSG], bf16, kind="ExternalInput")
    Wbil_d = nc.dram_tensor("Wbil", [128, 2, D], f8, kind="ExternalInput")
    Wr1a_d = nc.dram_tensor("Wr1a", [D, H2], bf16, kind="ExternalInput")
    Wr1c_d = nc.dram_tensor("Wr1c", [MSG + 1, H2], bf16,
                            kind="ExternalInput")
    Wr2_d = nc.dram_tensor("Wr2", [H2, D], bf16, kind="ExternalInput")
    b1_d = nc.dram_tensor("b1", [H1], f32, kind="ExternalInput")
    out_d = nc.dram_tensor("out", [D, bpc, N], bf16, kind="ExternalOutput")

    with tile.TileContext(nc) as tc, ExitStack() as ctx:
        wp = ctx.enter_context(tc.tile_pool(name="wp", bufs=1))
        dp = ctx.enter_context(tc.tile_pool(name="dp", bufs=5))
        sp = ctx.enter_context(tc.tile_pool(name="sp", bufs=3))
        pp = ctx.enter_context(tc.tile_pool(name="pp", bufs=1, space="PSUM"))

        # ---------------- one-time setup ----------------
        ident = wp.tile([128, 128], f32)
        make_identity(nc, ident[:])
        ident_b = wp.tile([128, 128], bf16)
        nc.vector.tensor_copy(ident_b[:], ident[:])

        def loadw(dram_ap, shape, name, dtype=bf16, eng=nc.scalar):
            t = wp.tile(shape, dtype, name=name)
            eng.dma_start(t[:], dram_ap)
            return t

        W1_8 = loadw(W1_d[:], [128, 2, H1], "W1", f8)
        Wbil_8 = loadw(Wbil_d[:], [128, 2, D], "Wbil", f8, nc.gpsimd)
        Wf_b = loadw(Wf_d[:], [H1, MSG], "Wf")
        Wr1_r0 = loadw(Wr1a_d[0:128, :], [128, H2], "Wr1a", bf16, nc.gpsimd)
        Wr1_r1 = loadw(Wr1a_d[128:256, :], [128, H2], "Wr1b")
        Wr1c_b = loadw(Wr1c_d[:], [MSG + 1, H2], "Wr1c", bf16, nc.gpsimd)
        Wr2_r0 = loadw(Wr2_d[0:128, :], [128, D], "Wr2a")
        Wr2_r1 = loadw(Wr2_d[128:256, :], [128, D], "Wr2b", bf16, nc.gpsimd)

        b1_sb = wp.tile([H1, 1], f32, name="b1s")
        nc.scalar.dma_start(
            b1_sb[:], b1_d[:].rearrange("(p o) -> p o", o=1))

        # persistent aggT tiles with a constant ones-row (row MSG) for
        # the folded message bias (+ br1)
        aggT_tiles = []
        for i in range(2):
            t = wp.tile([MSG + 1, 2, N], bf16, name=f"aggTp{i}")
            nc.vector.memset(t[MSG:MSG + 1, :, :], 1.0)
            aggT_tiles.append(t)

        obsT_v = obsT_d[:].rearrange("(c d) b n -> d c b n", c=2)
        out_v = out_d[:].rearrange("(c d) b n -> d c b n", c=2)

        # ---------------- pipeline stages ----------------
        state = {}

        def emit_od(p):
            ob = dp.tile([128, 2, 2, N], bf16, name="od", tag="od", bufs=5)
            nc.sync.dma_start(ob[:], obsT_v[:, :, 2 * p:2 * p + 2, :])
            o8 = dp.tile([128, 2, 2, N], f8, name="od8", tag="od8", bufs=3)
            nc.sync.dma_start(o8[:], obs8_d[:, :, 2 * p:2 * p + 2, :])
            state[("od", p)] = ob
            state[("od8", p)] = o8

        def emit_ht(p):
            o8 = state[("od8", p)]
            hT_ps = pp.tile([H1, 2 * N], f32, tag="g", bufs=1)
            nc.tensor.matmul(hT_ps[:], W1_8[:],
                             o8[:].rearrange("d c b n -> d c (b n)"),
                             start=True, stop=True, perf_mode=DR)
            hT_b = sp.tile([H1, 2 * N], bf16, name="hT_b", tag="hT",
                           bufs=2)
            nc.scalar.activation(hT_b[:], hT_ps[:], AF.Relu, bias=b1_sb[:])
            state[("hT", p)] = hT_b

        def emit_tmp(p):
            o8 = state[("od8", p)]
            rhs = o8[:].rearrange("d c b n -> d c (b n)")
            tmp_ps = pp.tile([128, 2, 2 * N], f32, tag="a", bufs=1)
            for ec in range(2):
                nc.tensor.matmul(tmp_ps[:, ec, :],
                                 Wbil_8[:, :, 128 * ec:128 * (ec + 1)],
                                 rhs, start=True, stop=True, perf_mode=DR)
            tmpT8 = sp.tile([128, 2, 2 * N], f8, name="tmpT8",
                            tag="tmpT", bufs=2)
            nc.vector.tensor_copy(
                tmpT8[:].rearrange("e c f -> e (c f)"),
                tmp_ps[:].rearrange("e c f -> e (c f)"))
            state[("tmpT", p)] = tmpT8

        def emit_msn(p):
            hT_b = state.pop(("hT", p))
            msn_ps = pp.tile([128, 4, MSG], f32, tag="g", bufs=1,
                             name="msn_ps")
            for q in range(4):
                nc.tensor.matmul(msn_ps[:, q, :],
                                 hT_b[:, 128 * q:128 * (q + 1)],
                                 Wf_b[:], start=True, stop=True)
            msgs_b = sp.tile([128, 4, MSG], bf16, name="msgs_b",
                             tag="msgs", bufs=3)
            nc.scalar.activation(
                msgs_b[:].rearrange("p q m -> p (q m)"),
                msn_ps[:].rearrange("p q m -> p (q m)"), AF.Copy)
            state[("msgs", p)] = msgs_b

        def emit_scores(p):
            o8 = state[("od8", p)]
            tmpT8 = state.pop(("tmpT", p))
            s_ps = pp.tile([128, 2, 2, N], f32, tag="b", bufs=1,
                           name="s_ps")
            for bi in range(2):
                boff = bi * N
                for ic in range(2):
                    ioff = boff + 128 * ic
                    nc.tensor.matmul(s_ps[:, bi, ic, :],
                                     tmpT8[:, :, ioff:ioff + 128],
                                     o8[:, :, bi, :],
                                     start=True, stop=True, perf_mode=DR)
            state[("s_ps", p)] = s_ps

        def emit_exp(p):
            s_ps = state.pop(("s_ps", p))
            E = sp.tile([128, 4, N], bf16, name="E", tag="E", bufs=4)
            nc.scalar.activation(
                E[:].rearrange("p c f -> p (c f)"),
                s_ps[:].rearrange("p b c f -> p (b c f)"), AF.Exp)
            state[("E", p)] = E

        def emit_gate(p):
            # top-8, den, rden, msk=(E>=t8)*rden on DVE; U = msk*E on
            # GPSIMD (its only fast path: plain tensor_tensor).
            E = state.pop(("E", p))
            top8 = sp.tile([128, 4, 8], f32, name="top8", tag="top8",
                           bufs=3)
            for c in range(4):
                nc.vector.max(out=top8[:, c, :], in_=E[:, c, :])
            den = sp.tile([128, 4], f32, name="den", tag="den", bufs=3)
            nc.vector.tensor_reduce(
                out=den[:], in_=top8[:], axis=mybir.AxisListType.X,
                op=OP.add)
            rden = sp.tile([128, 4], f32, name="rden", tag="rden", bufs=3)
            nc.vector.reciprocal(rden[:], den[:])
            msks = []
            for c in range(4):
                msk = sp.tile([128, N], bf16, name="msk", tag="msk",
                              bufs=6)
                nc.vector.tensor_scalar(
                    out=msk[:], in0=E[:, c, :],
                    scalar1=top8[:, c, 7:8],
                    scalar2=rden[:, c:c + 1],
                    op0=OP.is_ge, op1=OP.mult)
                msks.append(msk)
            Us = []
            for bi in range(2):
                U = sp.tile([128, 2, N], bf16, name="U", tag=f"U{bi}",
                            bufs=3)
                for ic in range(2):
                    c = 2 * bi + ic
                    nc.gpsimd.tensor_tensor(
                        out=U[:, ic, :], in0=msks[c],
                        in1=E[:, c, :], op=OP.mult)
                Us.append(U)
            state[("gate", p)] = Us

        def emit_gt(p):
            Us = state.pop(("gate", p))
            Gt_ps = pp.tile([128, 2, 2, N], f32, tag="c", bufs=1,
                            name="Gt_ps")
            for bi in range(2):
                U = Us[bi]
                for ic in range(2):
                    for jc in range(2):
                        nc.tensor.matmul(
                            Gt_ps[:, bi, jc, 128 * ic:128 * (ic + 1)],
                            U[:, ic, 128 * jc:128 * (jc + 1)],
                            ident_b[:], start=True, stop=True)
            Gt_b = sp.tile([128, 2, 2, N], bf16, name="Gt_b", tag="Gt",
                           bufs=2)
            nc.scalar.activation(
                Gt_b[:].rearrange("p b c f -> p (b c f)"),
                Gt_ps[:].rearrange("p b c f -> p (b c f)"), AF.Copy)
            state[("Gt", p)] = Gt_b

        def emit_agg(p):
            Gt_b = state.pop(("Gt", p))
            msgs_b = state.pop(("msgs", p))
            aggT_ps = pp.tile([MSG, 2, N], f32, tag="e", bufs=1,
                              name="aggT_ps")
            for bi in range(2):
                nc.tensor.matmul(aggT_ps[:, bi, :],
                                 msgs_b[:, 2 * bi, :], Gt_b[:, bi, 0, :],
                                 start=True, stop=False)
                nc.tensor.matmul(aggT_ps[:, bi, :],
                                 msgs_b[:, 2 * bi + 1, :],
                                 Gt_b[:, bi, 1, :],
                                 start=False, stop=True)
            state[("aggT_ps", p)] = aggT_ps

        def emit_agg_cp(p):
            aggT_ps = state.pop(("aggT_ps", p))
            aggT_r = aggT_tiles[p % 2]
            nc.vector.tensor_copy(
                aggT_r[0:MSG, :, :].rearrange("m b n -> m (b n)"),
                aggT_ps[:].rearrange("m b n -> m (b n)"))

        def emit_l1(p):
            ob = state[("od", p)]
            aggT_r = aggT_tiles[p % 2]
            r_ps = pp.tile([128, 2, 2 * N], f32, tag="a", bufs=1,
                           name="r_ps")
            aggT_ap = aggT_r[:].rearrange("m b n -> m (b n)")
            od0 = ob[:, 0].rearrange("d b n -> d (b n)")
            od1 = ob[:, 1].rearrange("d b n -> d (b n)")
            for mi in range(2):
                ms = 128 * mi
                nc.tensor.matmul(r_ps[:, mi, :], Wr1_r0[:, ms:ms + 128],
                                 od0, start=True, stop=False)
                nc.tensor.matmul(r_ps[:, mi, :], Wr1_r1[:, ms:ms + 128],
                                 od1, start=False, stop=False)
                nc.tensor.matmul(r_ps[:, mi, :], Wr1c_b[:, ms:ms + 128],
                                 aggT_ap, start=False, stop=True)
            state[("r_ps", p)] = r_ps

        def emit_relu_r(p):
            r_ps = state.pop(("r_ps", p))
            rT = sp.tile([128, 2, 2 * N], bf16, name="rT", tag="rT",
                         bufs=3)
            nc.scalar.activation(
                rT[:].rearrange("h c f -> h (c f)"),
                r_ps[:].rearrange("h c f -> h (c f)"), AF.Relu)
            state[("rT", p)] = rT

        def emit_l2(p):
            rT = state.pop(("rT", p))
            state.pop(("od", p))
            state.pop(("od8", p))
            o_ps = pp.tile([128, 2, 2 * N], f32, tag="b", bufs=1,
                           name="o_ps")
            for dc in range(2):
                ds = 128 * dc
                nc.tensor.matmul(o_ps[:, dc, :], Wr2_r0[:, ds:ds + 128],
                                 rT[:, 0, :], start=True, stop=False)
                nc.tensor.matmul(o_ps[:, dc, :], Wr2_r1[:, ds:ds + 128],
                                 rT[:, 1, :], start=False, stop=True)
            o_sb = sp.tile([128, 2, 2, N], bf16, name="o_sb", tag="o_sb",
                           bufs=2)
            nc.scalar.activation(
                o_sb[:].rearrange("d c b n -> d (c b n)"),
                o_ps[:].rearrange("d c f -> d (c f)"), AF.Copy)
            nc.sync.dma_start(out_v[:, :, 2 * p:2 * p + 2, :], o_sb[:])

        # ---------------- main pipeline loop ----------------
        # Per-iteration emission order fixes each engine's queue order:
        #   PE:     hT(v) tmp(v) l2(v-4) msn(v) Gt(v-2) s(v) l1(v-3)
        #           aggT(v-2)
        #   scalar: relu_h(v) out(v-4) msn-cp(v) Gt-cast(v-2)
        #           relu_r(v-3) exp(v)
        #   vector: tmpT(v) Max8/den/msk(v-1) aggT-cp(v-2)
        #   gpsimd: U-mult(v-1) x4
        for _ in range(passes):
            emit_od(0)
            for v in range(npairs + 4):
                if v + 1 < npairs:
                    emit_od(v + 1)
                if v < npairs:
                    emit_ht(v)
                    emit_tmp(v)
                if v >= 4:
                    emit_l2(v - 4)
                if v < npairs:
                    emit_msn(v)
                if 2 <= v < npairs + 2:
                    emit_gt(v - 2)
                if v < npairs:
                    emit_scores(v)
                if 3 <= v < npairs + 3:
                    emit_l1(v - 3)
                    emit_relu_r(v - 3)
                if v < npairs:
                    emit_exp(v)
                if 2 <= v < npairs + 2:
                    emit_agg(v - 2)
                if 1 <= v < npairs + 1:
                    emit_gate(v - 1)
                if 2 <= v < npairs + 2:
                    emit_agg_cp(v - 2)

    nc.compile()
    return nc


def _np_inputs_for_core(inputs, core, bpc=BPC):
    import ml_dtypes

    bf = ml_dtypes.bfloat16
    f8 = ml_dtypes.float8_e4m3
    obs = np.asarray(inputs["obs_all"], np.float32)
    lo = core * bpc
    obsT = np.ascontiguousarray(
        obs[lo:lo + bpc].transpose(2, 0, 1))            # [D, bpc, N] f32
    obs8 = np.ascontiguousarray(
        obsT.reshape(2, 128, bpc, N).transpose(1, 0, 2, 3))

    W1 = np.asarray(inputs["W1"], np.float32)
    W2 = np.asarray(inputs["W2"], np.float32)
    b2 = np.asarray(inputs["b2"], np.float32)
    Wc = np.asarray(inputs["Wc"], np.float32)
    bc = np.asarray(inputs["bc"], np.float32)
    Wd = np.asarray(inputs["Wd"], np.float32)
    bd = np.asarray(inputs["bd"], np.float32)
    Wr1 = np.asarray(inputs["Wr1"], np.float32)
    br1 = np.asarray(inputs["br1"], np.float32)
    Wbil = np.asarray(inputs["Wbil"], np.float32)

    Wf = (W2 @ Wc) @ Wd                              # [H1, MSG]
    bf_vec = (b2 @ Wc) @ Wd + bc @ Wd + bd           # [MSG]
    Wr1c = Wr1[D:D + MSG]                            # [MSG, H2]
    # ones-row carries the folded message bias AND br1
    Wr1c_aug = np.vstack([Wr1c, (bf_vec @ Wr1c + br1)[None, :]])

    return {
        "obsT": obsT.astype(bf),
        "obs8": obs8.astype(f8),
        "W1": np.ascontiguousarray(
            W1.reshape(2, 128, H1).transpose(1, 0, 2)).astype(f8),
        "Wf": Wf.astype(bf),
        "Wbil": np.ascontiguousarray(
            Wbil.reshape(2, 128, D).transpose(1, 0, 2)).astype(f8),
        "Wr1a": Wr1[0:D].astype(bf),
        "Wr1c": np.ascontiguousarray(Wr1c_aug).astype(bf),
        "Wr2": np.asarray(inputs["Wr2"], np.float32).astype(bf),
        "b1": np.asarray(inputs["b1"], np.float32),
    }


def _finish(outT, br2):
    # outT: [D, bpc, N] bf16 -> [bpc, N, D] f32 + br2
    return outT.astype(np.float32).transpose(1, 2, 0) + br2[None, None, :]


def kernel(**inputs):
    from concourse.bass_utils import run_bass_kernel_spmd

    if "prog" not in _CACHE:
        _CACHE["prog"] = build_program(BPC)
    nc = _CACHE["prog"]

    br2 = np.asarray(inputs["br2"], np.float32)
    core_ids = list(range(NCORES))
    in_maps = [_np_inputs_for_core(inputs, c) for c in core_ids]
    res = run_bass_kernel_spmd(nc, in_maps, core_ids)
    out = np.concatenate(
        [_finish(np.asarray(res.results[c]["out"]), br2)
         for c in core_ids], axis=0)
    return out.astype(np.float32)


# revision 13
# speedup vs baseline: 2.4855x; 1.0346x over previous
"""Trainium2 Bass kernel for nn_BandwidthConstrainedComm.

GNN message passing: per batch element, N=256 agents each generate a
message (MLP -> compress -> decompress), compute pairwise bilinear
relevance scores, pick top-K=8 senders (softmax gated), aggregate their
messages, and run a receiver MLP over [obs, agg].

Sharding: pure data parallel over batch B=128 -> 16 per core x 8 cores.

Design notes (v4 - 5-stage pipeline + fp8 DoubleRow front-end):
  - obs uploaded twice: bf16 [d,2(dc),b,n] for the receiver MLP (fp8
    there breaks tolerance) and fp8-e4m3 [d_lo,2(d_hi),b,n] for the
    message/score path (verified: max-err unchanged, the bf16 receiver
    path dominates).
  - h-MLP, bilinear tmp and scores run as fp8 DoubleRow matmuls
    (K=256 per instruction, ~2x fewer PE instructions, 1.44x rate).
    tmpT is cast PSUM->fp8 so scores' stationary operand is fp8 too.
  - W2@Wc@Wd fused into one [H1, MSG] matrix on the host; message bias
    (+ br1) folded into the receiver matmul via a ones-row in aggT and
    an extra host-precomputed row in Wr1c; br2 added on the host.
  - top-8 via DVE Max8; den=sum(top8) one reduce; msk=(E>=t8)*rden on
    DVE (2-scalar tensor_scalar); U = msk*E on GPSIMD tensor_tensor
    (the only fast Pool op); gate transpose Gt = U.T @ I by PE.
  - 5-stage pipeline: pre(v) | gate(v-1) | Gt+agg(v-2) | l1(v-3) |
    l2+out(v-4), with per-engine queue orders chosen so every
    cross-engine dependency has >= half an iteration of slack.
  - engine budget per pair: PE ~4us, scalar relu_h/out/msn/Gt-cast/
    relu_r/exp ~5.5us, vector tmpT/Max8/den/recip/msk/aggT ~5.3us,
    gpsimd 4 mults ~2.7us.
  - PSUM 16KB/partition exactly: g[2K]=hT/msn, a[4K]=tmp/r, b[4K]=o/s,
    c[4K]=Gt, e[2K]=aggT.
  - output written as bf16 [D, bpc, N] in one DMA, un-transposed and
    f32-cast on the host.
"""

import sys

sys.path.insert(0, "/opt/trn_rl_repo")

import numpy as np

# problem dims (hardcoded per contract)
B, N, D = 128, 256, 256
MSG, CD, K = 64, 32, 8
H1, H2 = 128, 256
NCORES = 8
BPC = B // NCORES  # batches per core

_CACHE = {}


def build_program(bpc=BPC, passes=1):
    import concourse.bacc as bacc
    import concourse.mybir as mybir
    import concourse.tile as tile
    from concourse.masks import make_identity
    from contextlib import ExitStack

    dt = mybir.dt
    f32, bf16, f8 = dt.float32, dt.bfloat16, dt.float8e4
    AF = mybir.ActivationFunctionType
    OP = mybir.AluOpType
    DR = mybir.MatmulPerfMode.DoubleRow

    assert bpc % 2 == 0
    npairs = bpc // 2

    nc = bacc.Bacc("TRN2", target_bir_lowering=False, debug=False,
                   num_devices=NCORES)

    obsT_d = nc.dram_tensor("obsT", [D, bpc, N], bf16, kind="ExternalInput")
    obs8_d = nc.dram_tensor("obs8", [128, 2, bpc, N], f8,
                            kind="ExternalInput")
    W1_d = nc.dram_tensor("W1", [128, 2, H1], f8, kind="ExternalInput")
    Wf_d = nc.dram_tensor("Wf", [H1, MSG], bf16, kind="ExternalInput")
    Wbil_d = nc.dram_tensor("Wbil", [128, 2, D], f8, kind="ExternalInput")
    Wr1a_d = nc.dram_tensor("Wr1a", [D, H2], bf16, kind="ExternalInput")
    Wr1c_d = nc.dram_tensor("Wr1c", [MSG + 1, H2], bf16,
                            kind="ExternalInput")
    Wr2_d = nc.dram_tensor("Wr2", [H2, D], bf16, kind="ExternalInput")
    b1_d = nc.dram_tensor("b1", [H1], f32, kind="ExternalInput")
    out_d = nc.dram_tensor("out", [D, bpc, N], bf16, kind="ExternalOutput")

    with tile.TileContext(nc) as tc, ExitStack() as ctx:
        wp = ctx.enter_context(tc.tile_pool(name="wp", bufs=1))
        dp = ctx.enter_context(tc.tile_pool(name="dp", bufs=5))
        sp = ctx.enter_context(tc.tile_pool(name="sp", bufs=3))
        pp = ctx.enter_context(tc.tile_pool(name="pp", bufs=1, space="PSUM"))

        # ---------------- one-time setup ----------------
        ident = wp.tile([128, 128], f32)
        make_identity(nc, ident[:])
        ident_b = wp.tile([128, 128], bf16)
        nc.vector.tensor_copy(ident_b[:], ident[:])

        def loadw(dram_ap, shape, name, dtype=bf16, eng=nc.scalar):
            t = wp.tile(shape, dtype, name=name)
            eng.dma_start(t[:], dram_ap)
            return t

        W1_8 = loadw(W1_d[:], [128, 2, H1], "W1", f8)
        Wbil_8 = loadw(Wbil_d[:], [128, 2, D], "Wbil", f8, nc.gpsimd)
        Wf_b = loadw(Wf_d[:], [H1, MSG], "Wf")
        Wr1_r0 = loadw(Wr1a_d[0:128, :], [128, H2], "Wr1a", bf16, nc.gpsimd)
        Wr1_r1 = loadw(Wr1a_d[128:256, :], [128, H2], "Wr1b")
        Wr1c_b = loadw(Wr1c_d[:], [MSG + 1, H2], "Wr1c", bf16, nc.gpsimd)
        Wr2_r0 = loadw(Wr2_d[0:128, :], [128, D], "Wr2a")
        Wr2_r1 = loadw(Wr2_d[128:256, :], [128, D], "Wr2b", bf16, nc.gpsimd)

        b1_sb = wp.tile([H1, 1], f32, name="b1s")
        nc.scalar.dma_start(
            b1_sb[:], b1_d[:].rearrange("(p o) -> p o", o=1))

        # persistent aggT tiles with a constant ones-row (row MSG) for
        # the folded message bias (+ br1)
        aggT_tiles = []
        for i in range(2):
            t = wp.tile([MSG + 1, 2, N], bf16, name=f"aggTp{i}")
            nc.vector.memset(t[MSG:MSG + 1, :, :], 1.0)
            aggT_tiles.append(t)

        obsT_v = obsT_d[:].rearrange("(c d) b n -> d c b n", c=2)
        out_v = out_d[:].rearrange("(c d) b n -> d c b n", c=2)

        # ---------------- pipeline stages ----------------
        state = {}

        def emit_od(p):
            o8 = dp.tile([128, 2, 2, N], f8, name="od8", tag="od8", bufs=3)
            nc.sync.dma_start(o8[:], obs8_d[:, :, 2 * p:2 * p + 2, :])
            ob = dp.tile([128, 2, 2, N], bf16, name="od", tag="od", bufs=5)
            nc.sync.dma_start(ob[:], obsT_v[:, :, 2 * p:2 * p + 2, :])
            state[("od", p)] = ob
            state[("od8", p)] = o8

        def emit_ht(p):
            o8 = state[("od8", p)]
            hT_ps = pp.tile([H1, 2 * N], f32, tag="g", bufs=1)
            nc.tensor.matmul(hT_ps[:], W1_8[:],
                             o8[:].rearrange("d c b n -> d c (b n)"),
                             start=True, stop=True, perf_mode=DR)
            hT_b = sp.tile([H1, 2 * N], bf16, name="hT_b", tag="hT",
                           bufs=2)
            nc.scalar.activation(hT_b[:], hT_ps[:], AF.Relu, bias=b1_sb[:])
            state[("hT", p)] = hT_b

        def emit_tmp(p):
            o8 = state[("od8", p)]
            rhs = o8[:].rearrange("d c b n -> d c (b n)")
            tmp_ps = pp.tile([128, 2, 2 * N], f32, tag="a", bufs=1)
            for ec in range(2):
                nc.tensor.matmul(tmp_ps[:, ec, :],
                                 Wbil_8[:, :, 128 * ec:128 * (ec + 1)],
                                 rhs, start=True, stop=True, perf_mode=DR)
            tmpT8 = sp.tile([128, 2, 2 * N], f8, name="tmpT8",
                            tag="tmpT", bufs=2)
            nc.vector.tensor_copy(
                tmpT8[:].rearrange("e c f -> e (c f)"),
                tmp_ps[:].rearrange("e c f -> e (c f)"))
            state[("tmpT", p)] = tmpT8

        def emit_msn(p):
            hT_b = state.pop(("hT", p))
            msn_ps = pp.tile([128, 4, MSG], f32, tag="g", bufs=1,
                             name="msn_ps")
            for q in range(4):
                nc.tensor.matmul(msn_ps[:, q, :],
                                 hT_b[:, 128 * q:128 * (q + 1)],
                                 Wf_b[:], start=True, stop=True)
            msgs_b = sp.tile([128, 4, MSG], bf16, name="msgs_b",
                             tag="msgs", bufs=3)
            nc.scalar.activation(
                msgs_b[:].rearrange("p q m -> p (q m)"),
                msn_ps[:].rearrange("p q m -> p (q m)"), AF.Copy)
            state[("msgs", p)] = msgs_b

        def emit_scores(p):
            o8 = state[("od8", p)]
            tmpT8 = state.pop(("tmpT", p))
            s_ps = pp.tile([128, 2, 2, N], f32, tag="b", bufs=1,
                           name="s_ps")
            for bi in range(2):
                boff = bi * N
                for ic in range(2):
                    ioff = boff + 128 * ic
                    nc.tensor.matmul(s_ps[:, bi, ic, :],
                                     tmpT8[:, :, ioff:ioff + 128],
                                     o8[:, :, bi, :],
                                     start=True, stop=True, perf_mode=DR)
            state[("s_ps", p)] = s_ps

        def emit_exp(p):
            s_ps = state.pop(("s_ps", p))
            E = sp.tile([128, 4, N], bf16, name="E", tag="E", bufs=4)
            nc.scalar.activation(
                E[:].rearrange("p c f -> p (c f)"),
                s_ps[:].rearrange("p b c f -> p (b c f)"), AF.Exp)
            state[("E", p)] = E

        def emit_gate(p):
            # top-8, den, rden, msk=(E>=t8)*rden on DVE; U = msk*E on
            # GPSIMD (its only fast path: plain tensor_tensor). For the
            # LAST pair everything runs per-chunk on DVE so the tail
            # drain is latency-, not handoff-, bound.
            last = (p == npairs - 1)
            E = state.pop(("E", p))
            top8 = sp.tile([128, 4, 8], f32, name="top8", tag="top8",
                           bufs=3)
            den = sp.tile([128, 4], f32, name="den", tag="den", bufs=3)
            rden = sp.tile([128, 4], f32, name="rden", tag="rden", bufs=3)
            if not last:
                for c in range(4):
                    nc.vector.max(out=top8[:, c, :], in_=E[:, c, :])
                nc.vector.tensor_reduce(
                    out=den[:], in_=top8[:], axis=mybir.AxisListType.X,
                    op=OP.add)
                nc.vector.reciprocal(rden[:], den[:])
            msks = []
            for c in range(4):
                if last:
                    nc.vector.max(out=top8[:, c, :], in_=E[:, c, :])
                    nc.vector.tensor_reduce(
                        out=den[:, c:c + 1], in_=top8[:, c, :],
                        axis=mybir.AxisListType.X, op=OP.add)
                    nc.vector.reciprocal(rden[:, c:c + 1],
                                         den[:, c:c + 1])
                msk = sp.tile([128, N], bf16, name="msk", tag="msk",
                              bufs=6)
                nc.vector.tensor_scalar(
                    out=msk[:], in0=E[:, c, :],
                    scalar1=top8[:, c, 7:8],
                    scalar2=rden[:, c:c + 1],
                    op0=OP.is_ge, op1=OP.mult)
                msks.append(msk)
            Us = []
            for bi in range(2):
                U = sp.tile([128, 2, N], bf16, name="U", tag=f"U{bi}",
                            bufs=3)
                for ic in range(2):
                    c = 2 * bi + ic
                    eng = nc.vector if last else nc.gpsimd
                    eng.tensor_tensor(
                        out=U[:, ic, :], in0=msks[c],
                        in1=E[:, c, :], op=OP.mult)
                Us.append(U)
            state[("gate", p)] = Us

        def emit_gt(p):
            Us = state.pop(("gate", p))
            Gt_ps = pp.tile([128, 2, 2, N], f32, tag="c", bufs=1,
                            name="Gt_ps")
            for bi in range(2):
                U = Us[bi]
                for ic in range(2):
                    for jc in range(2):
                        nc.tensor.matmul(
                            Gt_ps[:, bi, jc, 128 * ic:128 * (ic + 1)],
                            U[:, ic, 128 * jc:128 * (jc + 1)],
                            ident_b[:], start=True, stop=True)
            Gt_b = sp.tile([128, 2, 2, N], bf16, name="Gt_b", tag="Gt",
                           bufs=2)
            nc.scalar.activation(
                Gt_b[:].rearrange("p b c f -> p (b c f)"),
                Gt_ps[:].rearrange("p b c f -> p (b c f)"), AF.Copy)
            state[("Gt", p)] = Gt_b

        def emit_agg(p):
            Gt_b = state.pop(("Gt", p))
            msgs_b = state.pop(("msgs", p))
            aggT_ps = pp.tile([MSG, 2, N], f32, tag="e", bufs=1,
                              name="aggT_ps")
            for bi in range(2):
                nc.tensor.matmul(aggT_ps[:, bi, :],
                                 msgs_b[:, 2 * bi, :], Gt_b[:, bi, 0, :],
                                 start=True, stop=False)
                nc.tensor.matmul(aggT_ps[:, bi, :],
                                 msgs_b[:, 2 * bi + 1, :],
                                 Gt_b[:, bi, 1, :],
                                 start=False, stop=True)
            state[("aggT_ps", p)] = aggT_ps

        def emit_agg_cp(p):
            aggT_ps = state.pop(("aggT_ps", p))
            aggT_r = aggT_tiles[p % 2]
            nc.vector.tensor_copy(
                aggT_r[0:MSG, :, :].rearrange("m b n -> m (b n)"),
                aggT_ps[:].rearrange("m b n -> m (b n)"))

        def emit_l1(p):
            ob = state[("od", p)]
            aggT_r = aggT_tiles[p % 2]
            r_ps = pp.tile([128, 2, 2 * N], f32, tag="a", bufs=1,
                           name="r_ps")
            aggT_ap = aggT_r[:].rearrange("m b n -> m (b n)")
            od0 = ob[:, 0].rearrange("d b n -> d (b n)")
            od1 = ob[:, 1].rearrange("d b n -> d (b n)")
            for mi in range(2):
                ms = 128 * mi
                nc.tensor.matmul(r_ps[:, mi, :], Wr1_r0[:, ms:ms + 128],
                                 od0, start=True, stop=False)
                nc.tensor.matmul(r_ps[:, mi, :], Wr1_r1[:, ms:ms + 128],
                                 od1, start=False, stop=False)
                nc.tensor.matmul(r_ps[:, mi, :], Wr1c_b[:, ms:ms + 128],
                                 aggT_ap, start=False, stop=True)
            state[("r_ps", p)] = r_ps

        def emit_relu_r(p):
            r_ps = state.pop(("r_ps", p))
            rT = sp.tile([128, 2, 2 * N], bf16, name="rT", tag="rT",
                         bufs=3)
            nc.scalar.activation(
                rT[:].rearrange("h c f -> h (c f)"),
                r_ps[:].rearrange("h c f -> h (c f)"), AF.Relu)
            state[("rT", p)] = rT

        def emit_l2(p):
            rT = state.pop(("rT", p))
            state.pop(("od", p))
            state.pop(("od8", p))
            o_ps = pp.tile([128, 2, 2 * N], f32, tag="b", bufs=1,
                           name="o_ps")
            for dc in range(2):
                ds = 128 * dc
                nc.tensor.matmul(o_ps[:, dc, :], Wr2_r0[:, ds:ds + 128],
                                 rT[:, 0, :], start=True, stop=False)
                nc.tensor.matmul(o_ps[:, dc, :], Wr2_r1[:, ds:ds + 128],
                                 rT[:, 1, :], start=False, stop=True)
            o_sb = sp.tile([128, 2, 2, N], bf16, name="o_sb", tag="o_sb",
                           bufs=2)
            nc.scalar.activation(
                o_sb[:].rearrange("d c b n -> d (c b n)"),
                o_ps[:].rearrange("d c f -> d (c f)"), AF.Copy)
            nc.sync.dma_start(out_v[:, :, 2 * p:2 * p + 2, :], o_sb[:])

        # ---------------- main pipeline loop ----------------
        # Per-iteration emission order fixes each engine's queue order:
        #   PE:     hT(v) tmp(v) l2(v-4) msn(v) Gt(v-2) s(v) l1(v-3)
        #           aggT(v-2)
        #   scalar: relu_h(v) out(v-4) msn-cp(v) Gt-cast(v-2)
        #           relu_r(v-3) exp(v)
        #   vector: tmpT(v) Max8/den/msk(v-1) aggT-cp(v-2)
        #   gpsimd: U-mult(v-1) x4
        for _ in range(passes):
            emit_od(0)
            for v in range(npairs + 4):
                if v + 1 < npairs:
                    emit_od(v + 1)
                if 1 <= v < npairs + 1:
                    emit_exp(v - 1)
                if v < npairs:
                    emit_ht(v)
                    emit_tmp(v)
                if v >= 4:
                    emit_l2(v - 4)
                if v < npairs:
                    emit_msn(v)
                if 2 <= v < npairs + 2:
                    emit_gt(v - 2)
                if v < npairs:
                    emit_scores(v)
                if 3 <= v < npairs + 3:
                    emit_l1(v - 3)
                    emit_relu_r(v - 3)
                if 1 <= v < npairs + 1:
                    emit_gate(v - 1)
                if 2 <= v < npairs + 2:
                    emit_agg(v - 2)
                if 2 <= v < npairs + 2:
                    emit_agg_cp(v - 2)

    nc.compile()
    return nc


def _np_inputs_for_core(inputs, core, bpc=BPC):
    import ml_dtypes

    bf = ml_dtypes.bfloat16
    f8 = ml_dtypes.float8_e4m3
    obs = np.asarray(inputs["obs_all"], np.float32)
    lo = core * bpc
    obsT = np.ascontiguousarray(
        obs[lo:lo + bpc].transpose(2, 0, 1))            # [D, bpc, N] f32
    obs8 = np.ascontiguousarray(
        obsT.reshape(2, 128, bpc, N).transpose(1, 0, 2, 3))

    W1 = np.asarray(inputs["W1"], np.float32)
    W2 = np.asarray(inputs["W2"], np.float32)
    b2 = np.asarray(inputs["b2"], np.float32)
    Wc = np.asarray(inputs["Wc"], np.float32)
    bc = np.asarray(inputs["bc"], np.float32)
    Wd = np.asarray(inputs["Wd"], np.float32)
    bd = np.asarray(inputs["bd"], np.float32)
    Wr1 = np.asarray(inputs["Wr1"], np.float32)
    br1 = np.asarray(inputs["br1"], np.float32)
    Wbil = np.asarray(inputs["Wbil"], np.float32)

    Wf = (W2 @ Wc) @ Wd                              # [H1, MSG]
    bf_vec = (b2 @ Wc) @ Wd + bc @ Wd + bd           # [MSG]
    Wr1c = Wr1[D:D + MSG]                            # [MSG, H2]
    # ones-row carries the folded message bias AND br1
    Wr1c_aug = np.vstack([Wr1c, (bf_vec @ Wr1c + br1)[None, :]])

    return {
        "obsT": obsT.astype(bf),
        "obs8": obs8.astype(f8),
        "W1": np.ascontiguousarray(
            W1.reshape(2, 128, H1).transpose(1, 0, 2)).astype(f8),
        "Wf": Wf.astype(bf),
        "Wbil": np.ascontiguousarray(
            Wbil.reshape(2, 128, D).transpose(1, 0, 2)).astype(f8),
        "Wr1a": Wr1[0:D].astype(bf),
        "Wr1c": np.ascontiguousarray(Wr1c_aug).astype(bf),
        "Wr2": np.asarray(inputs["Wr2"], np.float32).astype(bf),
        "b1": np.asarray(inputs["b1"], np.float32),
    }


def _finish(outT, br2):
    # outT: [D, bpc, N] bf16 -> [bpc, N, D] f32 + br2
    return outT.astype(np.float32).transpose(1, 2, 0) + br2[None, None, :]


def kernel(**inputs):
    from concourse.bass_utils import run_bass_kernel_spmd

    if "prog" not in _CACHE:
        _CACHE["prog"] = build_program(BPC)
    nc = _CACHE["prog"]

    br2 = np.asarray(inputs["br2"], np.float32)
    core_ids = list(range(NCORES))
    in_maps = [_np_inputs_for_core(inputs, c) for c in core_ids]
    res = run_bass_kernel_spmd(nc, in_maps, core_ids)
    out = np.concatenate(
        [_finish(np.asarray(res.results[c]["out"]), br2)
         for c in core_ids], axis=0)
    return out.astype(np.float32)


# revision 14
# speedup vs baseline: 2.7389x; 1.1020x over previous
"""Trainium2 Bass kernel for nn_BandwidthConstrainedComm.

GNN message passing: per batch element, N=256 agents each generate a
message (MLP -> compress -> decompress), compute pairwise bilinear
relevance scores, pick top-K=8 senders (softmax gated), aggregate their
messages, and run a receiver MLP over [obs, agg].

Sharding: pure data parallel over batch B=128 -> 16 per core x 8 cores.

Design notes (v4 - 5-stage pipeline + fp8 DoubleRow front-end):
  - obs uploaded twice: bf16 [d,2(dc),b,n] for the receiver MLP (fp8
    there breaks tolerance) and fp8-e4m3 [d_lo,2(d_hi),b,n] for the
    message/score path (verified: max-err unchanged, the bf16 receiver
    path dominates).
  - h-MLP, bilinear tmp and scores run as fp8 DoubleRow matmuls
    (K=256 per instruction, ~2x fewer PE instructions, 1.44x rate).
    tmpT is cast PSUM->fp8 so scores' stationary operand is fp8 too.
  - W2@Wc@Wd fused into one [H1, MSG] matrix on the host; message bias
    (+ br1) folded into the receiver matmul via a ones-row in aggT and
    an extra host-precomputed row in Wr1c; br2 added on the host.
  - top-8 via DVE Max8; den=sum(top8) one reduce; msk=(E>=t8)*rden on
    DVE (2-scalar tensor_scalar); U = msk*E on GPSIMD tensor_tensor
    (the only fast Pool op); gate transpose Gt = U.T @ I by PE.
  - 5-stage pipeline: pre(v) | gate(v-1) | Gt+agg(v-2) | l1(v-3) |
    l2+out(v-4), with per-engine queue orders chosen so every
    cross-engine dependency has >= half an iteration of slack.
  - engine budget per pair: PE ~4us, scalar relu_h/out/msn/Gt-cast/
    relu_r/exp ~5.5us, vector tmpT/Max8/den/recip/msk/aggT ~5.3us,
    gpsimd 4 mults ~2.7us.
  - PSUM 16KB/partition exactly: g[2K]=hT/msn, a[4K]=tmp/r, b[4K]=o/s,
    c[4K]=Gt, e[2K]=aggT.
  - output written as bf16 [D, bpc, N] in one DMA, un-transposed and
    f32-cast on the host.
"""

import sys

sys.path.insert(0, "/opt/trn_rl_repo")

import numpy as np

# problem dims (hardcoded per contract)
B, N, D = 128, 256, 256
MSG, CD, K = 64, 32, 8
H1, H2 = 128, 256
NCORES = 8
BPC = B // NCORES  # batches per core

_CACHE = {}


def build_program(bpc=BPC, passes=1):
    import concourse.bacc as bacc
    import concourse.mybir as mybir
    import concourse.tile as tile
    from concourse.masks import make_identity
    from contextlib import ExitStack

    dt = mybir.dt
    f32, bf16, f8 = dt.float32, dt.bfloat16, dt.float8e4
    AF = mybir.ActivationFunctionType
    OP = mybir.AluOpType
    DR = mybir.MatmulPerfMode.DoubleRow

    assert bpc % 2 == 0
    npairs = bpc // 2

    nc = bacc.Bacc("TRN2", target_bir_lowering=False, debug=False,
                   num_devices=NCORES)

    obsT_d = nc.dram_tensor("obsT", [D, bpc, N], bf16, kind="ExternalInput")
    obs8_d = nc.dram_tensor("obs8", [128, 2, bpc, N], f8,
                            kind="ExternalInput")
    W1_d = nc.dram_tensor("W1", [128, 2, H1], f8, kind="ExternalInput")
    Wf_d = nc.dram_tensor("Wf", [H1, MSG], bf16, kind="ExternalInput")
    Wbil_d = nc.dram_tensor("Wbil", [128, 2, D], f8, kind="ExternalInput")
    Wr1a_d = nc.dram_tensor("Wr1a", [D, H2], bf16, kind="ExternalInput")
    Wr1c_d = nc.dram_tensor("Wr1c", [MSG + 1, H2], bf16,
                            kind="ExternalInput")
    Wr2_d = nc.dram_tensor("Wr2", [H2, D], bf16, kind="ExternalInput")
    b1_d = nc.dram_tensor("b1", [H1], f32, kind="ExternalInput")
    out_d = nc.dram_tensor("out", [D, bpc, N], bf16, kind="ExternalOutput")

    with tile.TileContext(nc) as tc, ExitStack() as ctx:
        wp = ctx.enter_context(tc.tile_pool(name="wp", bufs=1))
        dp = ctx.enter_context(tc.tile_pool(name="dp", bufs=5))
        sp = ctx.enter_context(tc.tile_pool(name="sp", bufs=3))
        pp = ctx.enter_context(tc.tile_pool(name="pp", bufs=1, space="PSUM"))

        # ---------------- one-time setup ----------------
        ident = wp.tile([128, 128], f32)
        make_identity(nc, ident[:])
        ident_b = wp.tile([128, 128], bf16)
        nc.vector.tensor_copy(ident_b[:], ident[:])

        def loadw(dram_ap, shape, name, dtype=bf16, eng=nc.scalar):
            t = wp.tile(shape, dtype, name=name)
            eng.dma_start(t[:], dram_ap)
            return t

        W1_8 = loadw(W1_d[:], [128, 2, H1], "W1", f8)
        Wbil_8 = loadw(Wbil_d[:], [128, 2, D], "Wbil", f8, nc.gpsimd)
        Wf_b = loadw(Wf_d[:], [H1, MSG], "Wf")
        Wr1_r0 = loadw(Wr1a_d[0:128, :], [128, H2], "Wr1a", bf16, nc.gpsimd)
        Wr1_r1 = loadw(Wr1a_d[128:256, :], [128, H2], "Wr1b")
        Wr1c_b = loadw(Wr1c_d[:], [MSG + 1, H2], "Wr1c", bf16, nc.gpsimd)
        Wr2_r0 = loadw(Wr2_d[0:128, :], [128, D], "Wr2a")
        Wr2_r1 = loadw(Wr2_d[128:256, :], [128, D], "Wr2b", bf16, nc.gpsimd)

        b1_sb = wp.tile([H1, 1], f32, name="b1s")
        nc.scalar.dma_start(
            b1_sb[:], b1_d[:].rearrange("(p o) -> p o", o=1))

        # persistent aggT tiles with a constant ones-row (row MSG) for
        # the folded message bias (+ br1)
        aggT_tiles = []
        for i in range(2):
            t = wp.tile([MSG + 1, 2, N], bf16, name=f"aggTp{i}")
            nc.vector.memset(t[MSG:MSG + 1, :, :], 1.0)
            aggT_tiles.append(t)

        obsT_v = obsT_d[:].rearrange("(c d) b n -> d c b n", c=2)
        out_v = out_d[:].rearrange("(c d) b n -> d c b n", c=2)

        # ---------------- pipeline stages ----------------
        state = {}

        def emit_od(p):
            o8 = dp.tile([128, 2, 2, N], f8, name="od8", tag="od8", bufs=3)
            nc.sync.dma_start(o8[:], obs8_d[:, :, 2 * p:2 * p + 2, :])
            ob = dp.tile([128, 2, 2, N], bf16, name="od", tag="od", bufs=5)
            nc.sync.dma_start(ob[:], obsT_v[:, :, 2 * p:2 * p + 2, :])
            state[("od", p)] = ob
            state[("od8", p)] = o8

        def emit_ht(p):
            o8 = state[("od8", p)]
            hT_ps = pp.tile([H1, 2 * N], f32, tag="g", bufs=1)
            nc.tensor.matmul(hT_ps[:], W1_8[:],
                             o8[:].rearrange("d c b n -> d c (b n)"),
                             start=True, stop=True, perf_mode=DR)
            hT_b = sp.tile([H1, 2 * N], bf16, name="hT_b", tag="hT",
                           bufs=3)
            nc.scalar.activation(hT_b[:], hT_ps[:], AF.Relu, bias=b1_sb[:])
            state[("hT", p)] = hT_b

        def emit_tmp(p):
            o8 = state[("od8", p)]
            rhs = o8[:].rearrange("d c b n -> d c (b n)")
            tmp_ps = pp.tile([128, 2, 2 * N], f32, tag="a", bufs=1)
            for ec in range(2):
                nc.tensor.matmul(tmp_ps[:, ec, :],
                                 Wbil_8[:, :, 128 * ec:128 * (ec + 1)],
                                 rhs, start=True, stop=True, perf_mode=DR)
            tmpT8 = sp.tile([128, 2, 2 * N], f8, name="tmpT8",
                            tag="tmpT", bufs=3)
            nc.vector.tensor_copy(
                tmpT8[:].rearrange("e c f -> e (c f)"),
                tmp_ps[:].rearrange("e c f -> e (c f)"))
            state[("tmpT", p)] = tmpT8

        def emit_msn(p):
            hT_b = state.pop(("hT", p))
            msn_ps = pp.tile([128, 4, MSG], f32, tag="g", bufs=1,
                             name="msn_ps")
            for q in range(4):
                nc.tensor.matmul(msn_ps[:, q, :],
                                 hT_b[:, 128 * q:128 * (q + 1)],
                                 Wf_b[:], start=True, stop=True)
            msgs_b = sp.tile([128, 4, MSG], bf16, name="msgs_b",
                             tag="msgs", bufs=4)
            nc.scalar.activation(
                msgs_b[:].rearrange("p q m -> p (q m)"),
                msn_ps[:].rearrange("p q m -> p (q m)"), AF.Copy)
            state[("msgs", p)] = msgs_b

        def emit_scores(p):
            o8 = state[("od8", p)]
            tmpT8 = state.pop(("tmpT", p))
            s_ps = pp.tile([128, 2, 2, N], f32, tag="b", bufs=1,
                           name="s_ps")
            for bi in range(2):
                boff = bi * N
                for ic in range(2):
                    ioff = boff + 128 * ic
                    nc.tensor.matmul(s_ps[:, bi, ic, :],
                                     tmpT8[:, :, ioff:ioff + 128],
                                     o8[:, :, bi, :],
                                     start=True, stop=True, perf_mode=DR)
            state[("s_ps", p)] = s_ps

        def emit_exp(p):
            s_ps = state.pop(("s_ps", p))
            E = sp.tile([128, 4, N], bf16, name="E", tag="E", bufs=5)
            nc.scalar.activation(
                E[:].rearrange("p c f -> p (c f)"),
                s_ps[:].rearrange("p b c f -> p (b c f)"), AF.Exp)
            state[("E", p)] = E

        def emit_gate(p):
            # top-8 / den / rden / msk on DVE, processed in TWO
            # half-batches so the GPSIMD U-mults (which pace the Gt
            # matmuls two iterations later) start ~2.5us earlier.
            # For the LAST pair everything runs on DVE per-chunk so the
            # tail drain is latency-bound, not handoff-bound.
            last = (p == npairs - 1)
            E = state.pop(("E", p))
            top8 = sp.tile([128, 4, 8], f32, name="top8", tag="top8",
                           bufs=4)
            den = sp.tile([128, 4], f32, name="den", tag="den", bufs=4)
            rden = sp.tile([128, 4], f32, name="rden", tag="rden", bufs=4)
            Us = [sp.tile([128, 2, N], bf16, name="U", tag=f"U{bi}",
                          bufs=4) for bi in range(2)]
            for bi in range(2):
                for ic in range(2):
                    c = 2 * bi + ic
                    nc.vector.max(out=top8[:, c, :], in_=E[:, c, :])
                nc.vector.tensor_reduce(
                    out=den[:, 2 * bi:2 * bi + 2],
                    in_=top8[:, 2 * bi:2 * bi + 2, :],
                    axis=mybir.AxisListType.X, op=OP.add)
                nc.vector.reciprocal(rden[:, 2 * bi:2 * bi + 2],
                                     den[:, 2 * bi:2 * bi + 2])
                for ic in range(2):
                    c = 2 * bi + ic
                    msk = sp.tile([128, N], bf16, name="msk", tag="msk",
                                  bufs=8)
                    nc.vector.tensor_scalar(
                        out=msk[:], in0=E[:, c, :],
                        scalar1=top8[:, c, 7:8],
                        scalar2=rden[:, c:c + 1],
                        op0=OP.is_ge, op1=OP.mult)
                    eng = nc.vector if last else nc.gpsimd
                    eng.tensor_tensor(
                        out=Us[bi][:, ic, :], in0=msk[:],
                        in1=E[:, c, :], op=OP.mult)
            state[("gate", p)] = Us

        def emit_gt(p):
            Us = state.pop(("gate", p))
            Gt_ps = pp.tile([128, 2, 2, N], f32, tag="c", bufs=1,
                            name="Gt_ps")
            for bi in range(2):
                U = Us[bi]
                for ic in range(2):
                    for jc in range(2):
                        nc.tensor.matmul(
                            Gt_ps[:, bi, jc, 128 * ic:128 * (ic + 1)],
                            U[:, ic, 128 * jc:128 * (jc + 1)],
                            ident_b[:], start=True, stop=True)
            Gt_b = sp.tile([128, 2, 2, N], bf16, name="Gt_b", tag="Gt",
                           bufs=3)
            nc.scalar.activation(
                Gt_b[:].rearrange("p b c f -> p (b c f)"),
                Gt_ps[:].rearrange("p b c f -> p (b c f)"), AF.Copy)
            state[("Gt", p)] = Gt_b

        def emit_agg(p):
            Gt_b = state.pop(("Gt", p))
            msgs_b = state.pop(("msgs", p))
            aggT_ps = pp.tile([MSG, 2, N], f32, tag="e", bufs=1,
                              name="aggT_ps")
            for bi in range(2):
                nc.tensor.matmul(aggT_ps[:, bi, :],
                                 msgs_b[:, 2 * bi, :], Gt_b[:, bi, 0, :],
                                 start=True, stop=False)
                nc.tensor.matmul(aggT_ps[:, bi, :],
                                 msgs_b[:, 2 * bi + 1, :],
                                 Gt_b[:, bi, 1, :],
                                 start=False, stop=True)
            state[("aggT_ps", p)] = aggT_ps

        def emit_agg_cp(p):
            aggT_ps = state.pop(("aggT_ps", p))
            aggT_r = aggT_tiles[p % 2]
            nc.vector.tensor_copy(
                aggT_r[0:MSG, :, :].rearrange("m b n -> m (b n)"),
                aggT_ps[:].rearrange("m b n -> m (b n)"))

        def emit_l1(p):
            ob = state[("od", p)]
            aggT_r = aggT_tiles[p % 2]
            r_ps = pp.tile([128, 2, 2 * N], f32, tag="a", bufs=1,
                           name="r_ps")
            aggT_ap = aggT_r[:].rearrange("m b n -> m (b n)")
            od0 = ob[:, 0].rearrange("d b n -> d (b n)")
            od1 = ob[:, 1].rearrange("d b n -> d (b n)")
            for mi in range(2):
                ms = 128 * mi
                nc.tensor.matmul(r_ps[:, mi, :], Wr1_r0[:, ms:ms + 128],
                                 od0, start=True, stop=False)
                nc.tensor.matmul(r_ps[:, mi, :], Wr1_r1[:, ms:ms + 128],
                                 od1, start=False, stop=False)
                nc.tensor.matmul(r_ps[:, mi, :], Wr1c_b[:, ms:ms + 128],
                                 aggT_ap, start=False, stop=True)
            state[("r_ps", p)] = r_ps

        def emit_relu_r(p):
            r_ps = state.pop(("r_ps", p))
            rT = sp.tile([128, 2, 2 * N], bf16, name="rT", tag="rT",
                         bufs=4)
            nc.scalar.activation(
                rT[:].rearrange("h c f -> h (c f)"),
                r_ps[:].rearrange("h c f -> h (c f)"), AF.Relu)
            state[("rT", p)] = rT

        def emit_l2(p):
            rT = state.pop(("rT", p))
            state.pop(("od", p))
            state.pop(("od8", p))
            o_ps = pp.tile([128, 2, 2 * N], f32, tag="b", bufs=1,
                           name="o_ps")
            for dc in range(2):
                ds = 128 * dc
                nc.tensor.matmul(o_ps[:, dc, :], Wr2_r0[:, ds:ds + 128],
                                 rT[:, 0, :], start=True, stop=False)
                nc.tensor.matmul(o_ps[:, dc, :], Wr2_r1[:, ds:ds + 128],
                                 rT[:, 1, :], start=False, stop=True)
            o_sb = sp.tile([128, 2, 2, N], bf16, name="o_sb", tag="o_sb",
                           bufs=3)
            nc.scalar.activation(
                o_sb[:].rearrange("d c b n -> d (c b n)"),
                o_ps[:].rearrange("d c f -> d (c f)"), AF.Copy)
            nc.sync.dma_start(out_v[:, :, 2 * p:2 * p + 2, :], o_sb[:])

        # ---------------- main pipeline loop ----------------
        # Per-iteration emission order fixes each engine's queue order:
        #   PE:     hT(v) tmp(v) l2(v-4) msn(v) Gt(v-2) s(v) l1(v-3)
        #           aggT(v-2)
        #   scalar: relu_h(v) out(v-4) msn-cp(v) Gt-cast(v-2)
        #           relu_r(v-3) exp(v)
        #   vector: tmpT(v) Max8/den/msk(v-1) aggT-cp(v-2)
        #   gpsimd: U-mult(v-1) x4
        for _ in range(passes):
            emit_od(0)
            for v in range(npairs + 4):
                if v + 1 < npairs:
                    emit_od(v + 1)
                if 1 <= v < npairs + 1:
                    emit_exp(v - 1)
                if v < npairs:
                    emit_ht(v)
                    emit_tmp(v)
                if v >= 4:
                    emit_l2(v - 4)
                if v < npairs:
                    emit_msn(v)
                if 2 <= v < npairs + 2:
                    emit_gt(v - 2)
                if v < npairs:
                    emit_scores(v)
                if 3 <= v < npairs + 3:
                    emit_l1(v - 3)
                    emit_relu_r(v - 3)
                if 1 <= v < npairs + 1:
                    emit_gate(v - 1)
                if 2 <= v < npairs + 2:
                    emit_agg(v - 2)
                if 2 <= v < npairs + 2:
                    emit_agg_cp(v - 2)

    nc.compile()
    return nc


def _np_inputs_for_core(inputs, core, bpc=BPC):
    import ml_dtypes

    bf = ml_dtypes.bfloat16
    f8 = ml_dtypes.float8_e4m3
    obs = np.asarray(inputs["obs_all"], np.float32)
    lo = core * bpc
    obsT = np.ascontiguousarray(
        obs[lo:lo + bpc].transpose(2, 0, 1))            # [D, bpc, N] f32
    obs8 = np.ascontiguousarray(
        obsT.reshape(2, 128, bpc, N).transpose(1, 0, 2, 3))

    W1 = np.asarray(inputs["W1"], np.float32)
    W2 = np.asarray(inputs["W2"], np.float32)
    b2 = np.asarray(inputs["b2"], np.float32)
    Wc = np.asarray(inputs["Wc"], np.float32)
    bc = np.asarray(inputs["bc"], np.float32)
    Wd = np.asarray(inputs["Wd"], np.float32)
    bd = np.asarray(inputs["bd"], np.float32)
    Wr1 = np.asarray(inputs["Wr1"], np.float32)
    br1 = np.asarray(inputs["br1"], np.float32)
    Wbil = np.asarray(inputs["Wbil"], np.float32)

    Wf = (W2 @ Wc) @ Wd                              # [H1, MSG]
    bf_vec = (b2 @ Wc) @ Wd + bc @ Wd + bd           # [MSG]
    Wr1c = Wr1[D:D + MSG]                            # [MSG, H2]
    # ones-row carries the folded message bias AND br1
    Wr1c_aug = np.vstack([Wr1c, (bf_vec @ Wr1c + br1)[None, :]])

    return {
        "obsT": obsT.astype(bf),
        "obs8": obs8.astype(f8),
        "W1": np.ascontiguousarray(
            W1.reshape(2, 128, H1).transpose(1, 0, 2)).astype(f8),
        "Wf": Wf.astype(bf),
        "Wbil": np.ascontiguousarray(
            Wbil.reshape(2, 128, D).transpose(1, 0, 2)).astype(f8),
        "Wr1a": Wr1[0:D].astype(bf),
        "Wr1c": np.ascontiguousarray(Wr1c_aug).astype(bf),
        "Wr2": np.asarray(inputs["Wr2"], np.float32).astype(bf),
        "b1": np.asarray(inputs["b1"], np.float32),
    }


def _finish(outT, br2):
    # outT: [D, bpc, N] bf16 -> [bpc, N, D] f32 + br2
    return outT.astype(np.float32).transpose(1, 2, 0) + br2[None, None, :]


def kernel(**inputs):
    from concourse.bass_utils import run_bass_kernel_spmd

    if "prog" not in _CACHE:
        _CACHE["prog"] = build_program(BPC)
    nc = _CACHE["prog"]

    br2 = np.asarray(inputs["br2"], np.float32)
    core_ids = list(range(NCORES))
    in_maps = [_np_inputs_for_core(inputs, c) for c in core_ids]
    res = run_bass_kernel_spmd(nc, in_maps, core_ids)
    out = np.concatenate(
        [_finish(np.asarray(res.results[c]["out"]), br2)
         for c in core_ids], axis=0)
    return out.astype(np.float32)
